# revision 13
# baseline (speedup 1.0000x reference)
"""Trainium2 Bass kernel for a single transformer decoder layer
(B=2, S=2048, E=2048, 16 heads, FFN 4x, causal attention, exact gelu,
two layernorms), distributed over 8 NeuronCores.

Sharding:
  - QKV + attention: tensor-parallel over heads (2 heads/core), zero comm.
  - One AllToAll per batch exchanges ctx slices ([head-slice, all tokens]
    -> [all heads, 256-token slice]); each core then runs the fc
    projection with the full Wfc plus LN1 + FFN (full W1/W2) + LN2 on its
    own 512-token slice (256 from each batch). Host concatenates the 8
    output slices.

Schedule: qkv(b0) -> attention(b0) -> fire a2a(b0) -> qkv(b1) ->
attention(b1) -> fire a2a(b1) -> fc -> LN1 -> FFN -> LN2. The first
all-to-all's rendezvous (which absorbs inter-core launch skew) overlaps
batch-1 qkv+attention instead of stalling the PE.

Everything on-chip stays transposed ([feature, token]) so biases and
layernorm gains are per-partition ops and no transposes are needed.
Matmuls run in float32r (~13-bit mantissa, bf16 speed at N>=512).
"""
import functools
import math

import ml_dtypes
import numpy as np

BF16NP = ml_dtypes.bfloat16

import concourse.bacc as bacc
import concourse.bass as bass
import concourse.mybir as mybir
import concourse.tile as tile
from concourse.bass_utils import run_bass_kernel_spmd

N_CORES = 8
P = 128
B, S, E = 2, 2048, 2048
T = B * S                   # 4096 tokens
NH, HD = 16, 128
FF = 4 * E                  # 8192
KE = E // P                 # 16 contraction chunks
CPC = 2 * HD                # 256 head-dim columns per core
TBLK = T // N_CORES         # 512 tokens per core after the all-to-all
EPS = 1e-5

F32 = mybir.dt.float32
F32R = mybir.dt.float32r
BF16 = mybir.dt.bfloat16

Identity = mybir.ActivationFunctionType.Identity
Copy = mybir.ActivationFunctionType.Copy
Exp = mybir.ActivationFunctionType.Exp
Gelu = mybir.ActivationFunctionType.Gelu
Sqrt = mybir.ActivationFunctionType.Sqrt
ADD = mybir.AluOpType.add
MULT = mybir.AluOpType.mult
SUB = mybir.AluOpType.subtract


def _ln_finish(nc, pool, psums, x_t, onesf, g_t, be_t, eps_t, out_t, tag,
               mu_ps=None, sq_ps=None, mu_sb=None, m2_sb=None,
               chunk_done=None, cs=slice(0, 512)):
    """Finish a layernorm over token columns `cs`. Stats arrive as raw-sum
    psums or as already-scaled SBUF tiles. Apply: two chunk-pair-wide DVE
    ops build (x - mu)*rstd, then the scalar engine applies the per-chunk
    gain/bias: out = in*g + be."""
    W = cs.stop - cs.start
    if mu_sb is None:
        mu_sb = pool.tile([1, 512], F32R, tag=f"{tag}_musb", bufs=1,
                          name=f"{tag}_musb")[:, 0:W]
        nc.scalar.activation(mu_sb, mu_ps[:, cs], Copy, scale=1.0 / E)
        m2_sb = pool.tile([1, 512], F32, tag=f"{tag}_m2sb", bufs=1,
                          name=f"{tag}_m2sb")[:, 0:W]
        nc.scalar.activation(m2_sb, sq_ps[:, cs], Copy, scale=1.0 / E)
    var = pool.tile([1, 512], F32, tag=f"{tag}_var", bufs=1,
                    name=f"{tag}_var")[:, 0:W]
    nc.vector.tensor_mul(var, mu_sb, mu_sb)
    nc.vector.tensor_sub(var, m2_sb, var)
    std = pool.tile([1, 512], F32, tag=f"{tag}_std", bufs=1,
                    name=f"{tag}_std")[:, 0:W]
    nc.scalar.activation(std, var, Sqrt, bias=eps_t[:])
    rstd = pool.tile([1, 512], F32, tag=f"{tag}_rstd", bufs=1,
                     name=f"{tag}_rstd")[:, 0:W]
    nc.vector.reciprocal_approx_fast(rstd, std)
    rstd_r = pool.tile([1, 512], F32R, tag=f"{tag}_rstdr", bufs=1,
                       name=f"{tag}_rstdr")[:, 0:W]
    musr_r = pool.tile([1, 512], F32R, tag=f"{tag}_musr", bufs=1,
                       name=f"{tag}_musr")[:, 0:W]
    with nc.allow_low_precision(reason="f32r ln broadcast operands"):
        nc.vector.tensor_copy(rstd_r, rstd)
        nc.vector.tensor_mul(musr_r, mu_sb, rstd)
    r_bc = psums.tile([P, 512], F32, tag=f"{tag}_rbc", bufs=1,
                      name=f"{tag}_rbc")[:, 0:W]
    nc.tensor.matmul(r_bc, onesf[0:1, :], rstd_r, start=True, stop=True)
    mr_bc = psums.tile([P, 512], F32, tag=f"{tag}_mrbc", bufs=1,
                       name=f"{tag}_mrbc")[:, 0:W]
    nc.tensor.matmul(mr_bc, onesf[0:1, :], musr_r, start=True, stop=True)
    for kp in range(0, KE, 2):
        m2v = pool.tile([P, 2, 512], F32, tag=f"{tag}_m1", bufs=2,
                        name=f"{tag}_m1")[:, :, 0:W]
        nc.vector.tensor_mul(
            m2v, x_t[:, kp:kp + 2, cs],
            r_bc.rearrange("p (j t) -> p j t", j=1).to_broadcast([P, 2, W]))
        nc.vector.tensor_sub(
            m2v, m2v,
            mr_bc.rearrange("p (j t) -> p j t", j=1).to_broadcast([P, 2, W]))
        for j in (0, 1):
            k = kp + j
            out_ap = out_t(k) if callable(out_t) else out_t[:, k, cs]
            nc.scalar.activation(out_ap, m2v[:, j, :], Identity,
                                 bias=be_t[:, k:k + 1],
                                 scale=g_t[:, k:k + 1])
            if chunk_done is not None:
                chunk_done(k, out_ap)


def _build_program():
    nc = bacc.Bacc("TRN2", target_bir_lowering=False, debug=False,
                   num_devices=N_CORES)

    # ---- per-core external inputs ----
    embT_d = nc.dram_tensor("embT", [E, T], BF16, kind="ExternalInput")
    embres_d = nc.dram_tensor("embres", [P, KE * TBLK], BF16, kind="ExternalInput")
    wq_d = nc.dram_tensor("wq", [P, KE * CPC], BF16, kind="ExternalInput")
    wk_d = nc.dram_tensor("wk", [P, KE * CPC], BF16, kind="ExternalInput")
    wv_d = nc.dram_tensor("wv", [P, KE * CPC], BF16, kind="ExternalInput")
    bqk_d = nc.dram_tensor("bqk", [P, 4], F32, kind="ExternalInput")  # bq|bk chunks
    bvbc_d = nc.dram_tensor("bvbc", [P, CPC], F32, kind="ExternalInput")
    wfc_d = nc.dram_tensor("wfc", [16, P, KE * P], BF16, kind="ExternalInput")
    vecs_d = nc.dram_tensor("vecs", [P, 6 * KE], F32, kind="ExternalInput")
    # vecs: [bfc | g1 | be1 | b2 | g2 | be2] each [P, KE]
    w1_d = nc.dram_tensor("w1", [64, P, KE * P], BF16, kind="ExternalInput")
    b1_d = nc.dram_tensor("b1", [P, 64], F32, kind="ExternalInput")
    w2_d = nc.dram_tensor("w2", [4, 16, P, 16 * P], BF16, kind="ExternalInput")
    mask_d = nc.dram_tensor("maskT", [P, 4 * 512], F32R, kind="ExternalInput")
    ones_d = nc.dram_tensor("onesblk", [P, P], BF16, kind="ExternalInput")
    onesf_d = nc.dram_tensor("onesfblk", [P, P], F32R, kind="ExternalInput")

    out_d = nc.dram_tensor("outp", [P, KE, TBLK], F32, kind="ExternalOutput")

    # ---- internal DRAM ----
    qT_d = nc.dram_tensor("qT_i", [CPC, T], BF16, kind="Internal")
    kT_d = nc.dram_tensor("kT_i", [CPC, T], BF16, kind="Internal")
    v_d = nc.dram_tensor("v_i", [T, CPC], F32R, kind="Internal")
    HB = TBLK // 2   # 256-token half-block
    a2a0in_d = nc.dram_tensor("a2a0in_i", [N_CORES, CPC, HB], BF16, kind="Internal")
    a2a0out_d = nc.dram_tensor("a2a0out_i", [N_CORES, CPC, HB], BF16, kind="Internal")
    a2a1in_d = nc.dram_tensor("a2a1in_i", [N_CORES, CPC, HB], BF16, kind="Internal")
    a2a1out_d = nc.dram_tensor("a2a1out_i", [N_CORES, CPC, HB], BF16, kind="Internal")

    with tile.TileContext(nc) as tc:
        with (
            tc.tile_pool(name="const", bufs=1) as cpool,
            tc.tile_pool(name="persist", bufs=1) as ppool,
        ):
            # the q-weight/embedding interleave below goes FIRST on the sync
            # queue so the first matmul can start ASAP; consts follow.
            ones = cpool.tile([P, P], BF16, name="ones")
            onesf = cpool.tile([P, P], F32R, name="onesf")
            mask_t = cpool.tile([P, 4, 512], F32R, name="mask_t")
            nc.scalar.dma_start(mask_t[:], mask_d[:].rearrange("p (f t) -> p f t", f=4))
            bqk_t = cpool.tile([P, 4], F32, name="bqk_t")
            nc.scalar.dma_start(bqk_t[:], bqk_d[:])
            bvbc_t = cpool.tile([P, CPC], F32, name="bvbc_t")
            nc.scalar.dma_start(bvbc_t[:], bvbc_d[:])
            vecs_t = cpool.tile([P, 6, KE], F32, name="vecs_t")
            nc.scalar.dma_start(vecs_t[:], vecs_d[:].rearrange("p (v k) -> p v k", v=6))
            b1_t = cpool.tile([P, 64], F32, name="b1_t")
            nc.scalar.dma_start(b1_t[:], b1_d[:])
            eps_t = cpool.tile([1, 1], F32, name="eps_t")
            nc.vector.memset(eps_t[:], EPS)

            bfc_t = vecs_t[:, 0, :]
            g1_t = vecs_t[:, 1, :]
            be1_t = vecs_t[:, 2, :]
            b2_t = vecs_t[:, 3, :]
            g2_t = vecs_t[:, 4, :]
            be2_t = vecs_t[:, 5, :]

            old_t = ppool.tile([P, KE, TBLK], BF16, name="old_t")   # LN1 output
            y_sb = ppool.tile([P, KE, TBLK], F32, name="y_sb")      # FFN accum
            wfc_t = ppool.tile([P, 8, KE, P], BF16, name="wfc_lo")
            ctxL0 = ppool.tile([P, KE, HB], BF16, name="ctxL0")
            ctxL1 = ppool.tile([P, KE, HB], BF16, name="ctxL1")

            # ================= Phase Q: q/k/v projections =================
            with tc.tile_pool(name="qw", bufs=1) as qw:
                wq_t = qw.tile([P, KE, CPC], BF16, name="wq_t")
                wk_t = qw.tile([P, KE, CPC], BF16, name="wk_t")
                wv_t = qw.tile([P, KE, CPC], BF16, name="wv_t")

                def qkv_phase(b, qio, qps, e_pre=None, first=False):
                    if first:
                        e_pre = qio.tile([P, KE, 512], BF16, tag="emb", bufs=2,
                                         name="e_t")
                        wqv = wq_d[:].rearrange("p (k m) -> p k m", k=KE)
                        e0v = embT_d[:, 0:512].rearrange("(k p) t -> p k t", p=P)
                        for kc in range(0, KE, 4):
                            nc.sync.dma_start(wq_t[:, kc:kc + 4], wqv[:, kc:kc + 4])
                            nc.sync.dma_start(e_pre[:, kc:kc + 4], e0v[:, kc:kc + 4])
                        nc.sync.dma_start(
                            wk_t[:], wk_d[:].rearrange("p (k m) -> p k m", k=KE))
                        nc.sync.dma_start(
                            wv_t[:], wv_d[:].rearrange("p (k m) -> p k m", k=KE))
                        nc.sync.dma_start(ones[:], ones_d[:])
                        nc.sync.dma_start(onesf[:], onesf_d[:])
                        # prefetch the fc weights on the scalar engine's DMA
                        # queue so they stream during Q/A instead of stalling F
                        for nb in range(8):
                            nc.scalar.dma_start(
                                wfc_t[:, nb],
                                wfc_d.ap()[nb].rearrange("p (k m) -> p k m", k=KE))

                    for tbl in range(4):
                        tb = 4 * b + tbl
                        if tbl == 0 and e_pre is not None:
                            e_t = e_pre
                        else:
                            e_t = qio.tile([P, KE, 512], BF16, tag="emb", bufs=2,
                                           name="e_t")
                            nc.sync.dma_start(
                                e_t[:],
                                embT_d[:, tb * 512:(tb + 1) * 512]
                                .rearrange("(k p) t -> p k t", p=P),
                            )
                        for wi, (wt, dst) in enumerate(((wq_t, qT_d), (wk_t, kT_d))):
                            for hc in range(2):
                                pqk = qps.tile([P, 512], F32, tag="pqk", bufs=3,
                                               name="pqk")
                                for k in range(KE):
                                    nc.tensor.matmul(
                                        pqk[:], wt[:, k, hc * P:(hc + 1) * P],
                                        e_t[:, k, :],
                                        start=(k == 0), stop=(k == KE - 1),
                                    )
                                st = qio.tile([P, 512], BF16, tag="qkst", bufs=4,
                                              name="st")
                                nc.scalar.activation(
                                    st[:], pqk[:], Identity,
                                    bias=bqk_t[:, 2 * wi + hc:2 * wi + hc + 1])
                                nc.sync.dma_start(
                                    dst.ap()[hc * P:(hc + 1) * P,
                                             tb * 512:(tb + 1) * 512],
                                    st[:])
                        for tt in range(4):
                            pv = qps.tile([P, CPC], F32, tag="pv", bufs=3, name="pv")
                            for k in range(KE):
                                nc.tensor.matmul(
                                    pv[:], e_t[:, k, tt * P:(tt + 1) * P], wv_t[:, k, :],
                                    start=(k == 0), stop=(k == KE - 1),
                                )
                            vst = qio.tile([P, CPC], F32R, tag="vst", bufs=4, name="vst")
                            with nc.allow_low_precision(reason="f32r v store"):
                                nc.vector.tensor_add(vst[:], pv[:], bvbc_t[:])
                            nc.sync.dma_start(
                                v_d.ap()[tb * 512 + tt * P: tb * 512 + (tt + 1) * P, :],
                                vst[:])

                # ============ Phase A: causal attention (per batch) ========
                # scoresT/ctxT per head, all transposed; softmax denom via
                # ones-matmul; sc emission pipelined 2 deep; per-q-tile
                # normalization deferred one q-tile so the PE never waits on
                # the DVE chain (except the last tile of the batch, finalized
                # immediately so the all-to-all fires as early as possible).
                def attn_phase(b, aio, asc, aps):
                    a2ain = a2a0in_d if b == 0 else a2a1in_d
                    pending = [None]

                    def finalize(st):
                        hc, qt, ctx_ps, l_ps = st
                        l_sb = asc.tile([1, 512], F32, tag="lsb", bufs=2,
                                        name="l_sb")
                        nc.vector.tensor_copy(l_sb[:], l_ps[:])
                        r_sb = asc.tile([1, 512], F32, tag="rsb", bufs=2,
                                        name="r_sb")
                        nc.vector.reciprocal_approx_fast(r_sb[:], l_sb[:])
                        r_r = asc.tile([1, 512], BF16, tag="rr", bufs=2, name="r_r")
                        with nc.allow_low_precision(reason="bf16 recip bcast"):
                            nc.vector.tensor_copy(r_r[:], r_sb[:])
                        rbc_ps = aps.tile([P, 512], F32, tag="sc", bufs=2,
                                          name="rbc_ps")
                        nc.tensor.matmul(rbc_ps[:], ones[0:1, :], r_r[:],
                                         start=True, stop=True)
                        ctx_sb = asc.tile([P, 512], F32, tag="ctxsb", bufs=2,
                                          name="ctx_sb")
                        # on the vector queue (not scalar) so the next tile's
                        # Exp is never queued behind this copy
                        nc.vector.tensor_copy(ctx_sb[:], ctx_ps[:])
                        ctx_f = asc.tile([P, 512], BF16, tag="ctxf", bufs=2,
                                         name="ctx_f")
                        with nc.allow_low_precision(reason="bf16 ctx for a2a"):
                            nc.vector.tensor_mul(ctx_f[:], ctx_sb[:], rbc_ps[:])
                        nc.sync.dma_start(
                            a2ain.ap()[2 * qt, hc * P:(hc + 1) * P, :],
                            ctx_f[:, 0:HB])
                        nc.sync.dma_start(
                            a2ain.ap()[2 * qt + 1, hc * P:(hc + 1) * P, :],
                            ctx_f[:, HB:])

                    def load_h(hc):
                        # split per 512-token block so each slice streams as
                        # soon as its qkv-phase stores land
                        q_t = aio.tile([P, S], BF16, tag="q", bufs=2, name="q_t")
                        k_t = aio.tile([P, S], BF16, tag="k", bufs=2, name="k_t")
                        v_t = aio.tile([P, 16, P], F32R, tag="v", bufs=2, name="v_t")
                        for tbl in range(4):
                            ts = slice(b * S + tbl * 512, b * S + (tbl + 1) * 512)
                            ls = slice(tbl * 512, (tbl + 1) * 512)
                            nc.gpsimd.dma_start(
                                q_t[:, ls], qT_d.ap()[hc * P:(hc + 1) * P, ts])
                            nc.gpsimd.dma_start(
                                k_t[:, ls], kT_d.ap()[hc * P:(hc + 1) * P, ts])
                            nc.gpsimd.dma_start(
                                v_t[:, 4 * tbl:4 * tbl + 4, :],
                                v_d.ap()[ts, hc * P:(hc + 1) * P]
                                .rearrange("(j p) d -> p j d", p=P),
                            )
                        return q_t, k_t, v_t

                    nxt = load_h(0)
                    for hc in range(2):
                        q_t, k_t, v_t = nxt
                        if hc == 0:
                            nxt = load_h(1)
                        for qt in range(4):
                            nkb = 4 * qt + 4
                            ctx_ps = aps.tile([P, 512], F32, tag="ctx", bufs=2,
                                              name="ctx_ps")
                            l_full = aps.tile([P, 512], F32, tag="lr", bufs=2,
                                              name="l_full")
                            l_ps = l_full[0:1, :]
                            ex_pairs = [None] * nkb
                            ex_tiles = [None] * nkb
                            sc_cur = [None]

                            def emit_sc(kb, qt=qt, k_t=k_t, q_t=q_t,
                                        ex_tiles=ex_tiles, sc_cur=sc_cur,
                                        ex_pairs=ex_pairs):
                                # kb-blocks are processed in pairs sharing one
                                # 2-bank psum tile and a single wide Exp.
                                # Causal mask is a 0/1 DVE multiply; the
                                # softmax denominator accumulates on the PE.
                                half = kb % 2
                                if half == 0:
                                    sc_cur[0] = aps.tile([P, 2, 512], F32,
                                                         tag="sc", bufs=2,
                                                         name="sc_ps")
                                sc_ps = sc_cur[0]
                                # causally-dead q columns of diagonal blocks
                                # are never written (stale psum is finite; the
                                # 0/1 mask multiply zeroes exp of it)
                                d = max(0, (kb - 4 * qt)) * P
                                nc.tensor.matmul(
                                    sc_ps[:, half, d:],
                                    k_t[:, kb * P:(kb + 1) * P],
                                    q_t[:, qt * 512 + d:(qt + 1) * 512],
                                    start=True, stop=True)
                                if half == 1:
                                    ex = asc.tile([P, 2, 512], F32R, tag="ex",
                                                  bufs=3, name="ex")
                                    ds = [max(0, kb - 1 - 4 * qt) * P,
                                          max(0, kb - 4 * qt) * P]
                                    any_diag = kb >= 4 * qt
                                    # per-half exp always: the first ctx
                                    # matmul only waits on half 0, and diag
                                    # blocks never read unwritten psum
                                    for hh in (0, 1):
                                        nc.scalar.activation(
                                            ex[:, hh, ds[hh]:],
                                            sc_ps[:, hh, ds[hh]:], Exp)
                                    with nc.allow_low_precision(reason="attn mask"):
                                        for hh in (0, 1):
                                            kbb = kb - 1 + hh
                                            dd = ds[hh]
                                            if kbb >= 4 * qt:
                                                # triangular mask on the
                                                # diagonal 128-col block only
                                                nc.vector.tensor_mul(
                                                    ex[:, hh, dd:dd + P],
                                                    ex[:, hh, dd:dd + P],
                                                    mask_t[:, kbb - 4 * qt,
                                                           dd:dd + P])
                                    ex_pairs[kb] = (ex, any_diag, ds)
                                    ex_tiles[kb - 1] = ex[:, 0, :]
                                    ex_tiles[kb] = ex[:, 1, :]

                            for w in range(min(4, nkb)):
                                emit_sc(w)
                            for kb in range(nkb):
                                if kb + 4 < nkb:
                                    emit_sc(kb + 4)
                                d = max(0, (kb - 4 * qt)) * P
                                nc.tensor.matmul(ctx_ps[:, d:], v_t[:, kb, :],
                                                 ex_tiles[kb][:, d:],
                                                 start=(kb == 0),
                                                 stop=(kb == nkb - 1))
                                ex_tiles[kb] = None
                                if kb % 2 == 1:
                                    # softmax denominator rides the PE,
                                    # clipped to the live range per half
                                    ex, any_diag, ds = ex_pairs[kb]
                                    last = kb == nkb - 1
                                    nc.tensor.matmul(
                                        l_ps[:, ds[0]:], onesf[:, 0:1],
                                        ex[:, 0, ds[0]:],
                                        start=(kb == 1), stop=False)
                                    nc.tensor.matmul(
                                        l_ps[:, ds[1]:], onesf[:, 0:1],
                                        ex[:, 1, ds[1]:],
                                        start=False, stop=last)
                                    ex_pairs[kb] = None
                            if pending[0] is not None:
                                finalize(pending[0])
                            pending[0] = (hc, qt, ctx_ps, l_ps)
                    # last tile: finalize immediately so the all-to-all for
                    # this batch fires as early as possible
                    finalize(pending[0])

                with (
                    tc.tile_pool(name="qio0", bufs=2) as qio0,
                    tc.tile_pool(name="qps0", bufs=1, space="PSUM") as qps0,
                ):
                    qkv_phase(0, qio0, qps0, first=True)
                # prefetch batch-1's first embedding block on the gpsimd
                # queue (idle after the attention loads) so qkv(b1) starts
                # the moment attention(b0) ends
                e_pre1 = qw.tile([P, KE, 512], BF16, name="e_pre1")
                with (
                    tc.tile_pool(name="aio0", bufs=2) as aio0,
                    tc.tile_pool(name="asc0", bufs=1) as asc0,
                    tc.tile_pool(name="aps0", bufs=1, space="PSUM") as aps0,
                ):
                    attn_phase(0, aio0, asc0, aps0)
                nc.gpsimd.dma_start(
                    e_pre1[:],
                    embT_d[:, 4 * 512:5 * 512].rearrange("(k p) t -> p k t", p=P))
                # batch-0 ctx fully written -> exchange it while batch-1
                # qkv + attention run.
                nc.gpsimd.collective_compute(
                    "AllToAll", mybir.AluOpType.bypass,
                    replica_groups=[list(range(N_CORES))],
                    ins=[a2a0in_d.ap()], outs=[a2a0out_d.ap()],
                )
                with (
                    tc.tile_pool(name="qio1", bufs=2) as qio1,
                    tc.tile_pool(name="qps1", bufs=1, space="PSUM") as qps1,
                ):
                    qkv_phase(1, qio1, qps1, e_pre=e_pre1)
                with (
                    tc.tile_pool(name="aio1", bufs=2) as aio1,
                    tc.tile_pool(name="asc1", bufs=1) as asc1,
                    tc.tile_pool(name="aps1", bufs=1, space="PSUM") as aps1,
                ):
                    attn_phase(1, aio1, asc1, aps1)
                # ctxL0 load sits AFTER batch-1's q/k/v loads on the gpsimd
                # queue so attention never queues behind a collective wait.
                nc.gpsimd.dma_start(
                    ctxL0[:],
                    a2a0out_d.ap().rearrange("r (c p) t -> p (r c) t", p=P))
                nc.gpsimd.collective_compute(
                    "AllToAll", mybir.AluOpType.bypass,
                    replica_groups=[list(range(N_CORES))],
                    ins=[a2a1in_d.ap()], outs=[a2a1out_d.ap()],
                )
                nc.gpsimd.dma_start(
                    ctxL1[:],
                    a2a1out_d.ap().rearrange("r (c p) t -> p (r c) t", p=P))

            # ====== Phase F: fc with full Wfc + residual + LN1 stats =======
            # nwp (FFN weight/hidden tiles) opens BEFORE fio so its tiles
            # don't alias x_t: the first w1 loads then stream during fc
            # instead of waiting for LN1 to release x_t's memory.
            with tc.tile_pool(name="nwp", bufs=1) as nwp:
                with (
                    tc.tile_pool(name="fio", bufs=1) as fio,
                    tc.tile_pool(name="fps", bufs=1, space="PSUM") as fps,
                ):
                    x_t = fio.tile([P, KE, TBLK], BF16, name="x_t")
                    mu_ps = fps.tile([1, 512], F32, tag="ln1_mu", bufs=1,
                                     name="ln1_mu")
                    sq_ps = fps.tile([1, 512], F32, tag="ln1_sq", bufs=1,
                                     name="ln1_sq")

                    def fc_stats(nb, cs):
                        nc.tensor.matmul(mu_ps[:, cs], ones[:, 0:1],
                                         x_t[:, nb, cs],
                                         start=(nb == 0), stop=(nb == 15))
                        sqk = fio.tile([P, HB], BF16, tag="sqk", bufs=3,
                                       name="sqk")
                        with nc.allow_low_precision(reason="bf16 ln1 squares"):
                            nc.vector.tensor_mul(sqk[:], x_t[:, nb, cs],
                                                 x_t[:, nb, cs])
                        nc.tensor.matmul(sq_ps[:, cs], ones[:, 0:1], sqk[:],
                                         start=(nb == 0), stop=(nb == 15))

                    wfc_hi = {}
                    for h in range(2):
                        cs = slice(h * HB, (h + 1) * HB)
                        ctxh = ctxL0 if h == 0 else ctxL1
                        for nb in range(16):
                            if nb < 8:
                                wnb = wfc_t[:, nb]
                            elif nb < 12:
                                # cached across both column halves
                                if h == 0:
                                    whi = fio.tile([P, KE, P], BF16,
                                                   tag="wfchi", bufs=4,
                                                   name="wfc_hi")
                                    nc.sync.dma_start(
                                        whi[:],
                                        wfc_d.ap()[nb].rearrange(
                                            "p (k m) -> p k m", k=KE))
                                    wfc_hi[nb] = whi
                                wnb = wfc_hi[nb][:]
                            else:
                                # re-streamed per half (SBUF pressure)
                                whi = fio.tile([P, KE, P], BF16, tag="wfcs",
                                               bufs=2, name="wfc_s")
                                nc.sync.dma_start(
                                    whi[:],
                                    wfc_d.ap()[nb].rearrange(
                                        "p (k m) -> p k m", k=KE))
                                wnb = whi[:]
                            embres_t = fio.tile([P, HB], BF16, tag="embres",
                                                bufs=3, name="embres_t")
                            nc.sync.dma_start(
                                embres_t[:],
                                embres_d[:, nb * TBLK + h * HB:
                                         nb * TBLK + (h + 1) * HB])
                            pfc = fps.tile([P, HB], F32, tag="pfc", bufs=3,
                                           name="pfc")
                            for k in range(KE):
                                nc.tensor.matmul(pfc[:], wnb[:, k, :],
                                                 ctxh[:, k, :],
                                                 start=(k == 0),
                                                 stop=(k == KE - 1))
                            with nc.allow_low_precision(reason="bf16 ln1 input"):
                                nc.vector.scalar_tensor_tensor(
                                    x_t[:, nb, cs], pfc[:], bfc_t[:, nb:nb + 1],
                                    embres_t[:], ADD, ADD)
                            if nb > 0:
                                fc_stats(nb - 1, cs)
                        fc_stats(15, cs)
                        _ln_finish(nc, fio, fps, x_t, onesf, g1_t, be1_t,
                                   eps_t, old_t, "ln1", mu_ps=mu_ps,
                                   sq_ps=sq_ps, cs=cs)

                # ======== Phase N: FFN with LN2 input + stats fused ========
                with tc.tile_pool(name="l2", bufs=1) as l2p:
                    x2_t = l2p.tile([P, KE, TBLK], F32R, name="x2_t")
                    mu2_sb = l2p.tile([1, 512], F32R, name="mu2_sb")
                    m22_sb = l2p.tile([1, 512], F32, name="m22_sb")
                    with tc.tile_pool(name="nps", bufs=1, space="PSUM") as nps:
                        mu2_ps = nps.tile([1, 512], F32, tag="ln2_mu", bufs=1,
                                          name="ln2_mu")
                        sq2_ps = nps.tile([1, 512], F32, tag="ln2_sq", bufs=1,
                                          name="ln2_sq")

                        class _HpsShim:
                            """_ln_finish psum allocator that reuses the idle
                            hps tag so ln2 can run inside the nps scope."""
                            def tile(self, shape, dtype, tag=None, bufs=1,
                                     name=None):
                                return nps.tile(shape, dtype, tag="hps",
                                                bufs=3, name=name)
                        hps_shim = _HpsShim()

                        def y_accum(hbg, nb, c2, h_t):
                            w2_t = nwp.tile([P, 16, P], BF16, tag="w2", bufs=2,
                                            name="w2_t")
                            nc.sync.dma_start(
                                w2_t[:],
                                w2_d.ap()[hbg, nb].rearrange(
                                    "p (l m) -> p l m", l=16))
                            yps = nps.tile([P, 512], F32, tag="yps", bufs=3,
                                           name="yps")[:, 0:c2.stop - c2.start]
                            for hl in range(16):
                                nc.tensor.matmul(yps, w2_t[:, hl, :],
                                                 h_t[:, hl, c2],
                                                 start=(hl == 0),
                                                 stop=(hl == 15))
                            return yps

                        for hbg in range(4):
                            h_t = nwp.tile([P, 16, TBLK], BF16, tag="h",
                                           bufs=1, name="h_t")
                            for hl in range(16):
                                hb = hbg * 16 + hl
                                w1_t = nwp.tile([P, KE, P], BF16, tag="w1",
                                                bufs=2, name="w1_t")
                                nc.sync.dma_start(
                                    w1_t[:],
                                    w1_d.ap()[hb].rearrange("p (k m) -> p k m",
                                                            k=KE))
                                hps = nps.tile([P, 512], F32, tag="hps",
                                               bufs=3, name="hps")
                                if hbg == 0 and hl < 4:
                                    # halves: start on the early-ready ln1-h0
                                    # columns while ln1-h1 is still applying
                                    for h2 in range(2):
                                        c2 = slice(h2 * HB, (h2 + 1) * HB)
                                        for k in range(KE):
                                            nc.tensor.matmul(
                                                hps[:, c2], w1_t[:, k, :],
                                                old_t[:, k, c2],
                                                start=(k == 0),
                                                stop=(k == KE - 1))
                                else:
                                    for k in range(KE):
                                        nc.tensor.matmul(hps[:], w1_t[:, k, :],
                                                         old_t[:, k, :],
                                                         start=(k == 0),
                                                         stop=(k == KE - 1))
                                nc.scalar.activation(h_t[:, hl, :], hps[:],
                                                     Gelu,
                                                     bias=b1_t[:, hb:hb + 1])
                            if hbg < 3:
                                for nb in range(16):
                                    yps = y_accum(hbg, nb, slice(0, 512), h_t)
                                    if hbg == 0:
                                        nc.vector.tensor_copy(y_sb[:, nb, :],
                                                              yps)
                                    else:
                                        nc.vector.tensor_add(y_sb[:, nb, :],
                                                             y_sb[:, nb, :],
                                                             yps)
                            else:
                                # last group: split by column half so ln2 on
                                # half 0 overlaps the half-1 matmuls
                                for h2 in range(2):
                                    c2 = slice(h2 * HB, (h2 + 1) * HB)
                                    for nb in range(16):
                                        yps = y_accum(hbg, nb, c2, h_t)
                                        nc.vector.tensor_add(y_sb[:, nb, c2],
                                                             y_sb[:, nb, c2],
                                                             yps)
                                        nc.vector.scalar_tensor_tensor(
                                            x2_t[:, nb, c2], y_sb[:, nb, c2],
                                            b2_t[:, nb:nb + 1],
                                            old_t[:, nb, c2], ADD, ADD)
                                        sq2 = nwp.tile([P, 512], F32R,
                                                       tag="sq2", bufs=1,
                                                       name="sq2")[:, 0:HB]
                                        nc.vector.tensor_mul(
                                            sq2, x2_t[:, nb, c2],
                                            x2_t[:, nb, c2])
                                        nc.tensor.matmul(mu2_ps[:, c2],
                                                         onesf[:, 0:1],
                                                         x2_t[:, nb, c2],
                                                         start=(nb == 0),
                                                         stop=(nb == 15))
                                        nc.tensor.matmul(sq2_ps[:, c2],
                                                         onesf[:, 0:1], sq2,
                                                         start=(nb == 0),
                                                         stop=(nb == 15))
                                    nc.scalar.activation(mu2_sb[:, c2],
                                                         mu2_ps[:, c2], Copy,
                                                         scale=1.0 / E)
                                    nc.scalar.activation(m22_sb[:, c2],
                                                         sq2_ps[:, c2], Copy,
                                                         scale=1.0 / E)
                                    _ln_finish(
                                        nc, l2p, hps_shim, x2_t, onesf, g2_t,
                                        be2_t, eps_t,
                                        lambda k: l2p.tile(
                                            [P, 512], F32, tag="osb", bufs=2,
                                            name="out_sb")[:, 0:HB],
                                        "ln2", mu_sb=mu2_sb[:, c2],
                                        m2_sb=m22_sb[:, c2],
                                        chunk_done=lambda k, ap, c2=c2:
                                            nc.sync.dma_start(
                                                out_d.ap()[:, k, c2], ap),
                                        cs=c2)

    nc.compile()
    return nc


@functools.lru_cache(maxsize=1)
def _get_program():
    return _build_program()


def _pack_w(w):
    """[E_rows, M] -> [128, (E_rows/128)*M] with [p, k, m] layout."""
    e, m = w.shape
    return np.ascontiguousarray(
        w.reshape(e // P, P, m).transpose(1, 0, 2).reshape(P, -1))


def _pack_vec(v):
    """[n*128] -> [128, n] per-partition chunks."""
    return np.ascontiguousarray(v.reshape(-1, P).T)


def _prepare_in_maps(inputs):
    f32 = np.float32
    emb = np.asarray(inputs["embeddings"], f32).reshape(T, E)
    embT = np.ascontiguousarray(emb.T.astype(BF16NP))
    scale = 1.0 / math.sqrt(HD)

    Wq = np.asarray(inputs["Wq"], f32)
    Wk = np.asarray(inputs["Wk"], f32)
    Wv = np.asarray(inputs["Wv"], f32)
    bq = np.asarray(inputs["bq"], f32)
    bk = np.asarray(inputs["bk"], f32)
    bv = np.asarray(inputs["bv"], f32)
    Wfc = np.asarray(inputs["Wfc"], f32)
    W1 = np.asarray(inputs["W1"], f32)
    W2 = np.asarray(inputs["W2"], f32)

    vecs = np.concatenate([
        _pack_vec(np.asarray(inputs[n], f32))
        for n in ("bfc", "g1", "be1", "b2", "g2", "be2")
    ], axis=1)  # [128, 6*KE]

    wfcp = np.ascontiguousarray(
        Wfc.reshape(KE, P, 16, P).transpose(2, 1, 0, 3).reshape(16, P, KE * P)
        .astype(BF16NP))
    w1p = np.ascontiguousarray(
        W1.reshape(KE, P, 64, P).transpose(2, 1, 0, 3).reshape(64, P, KE * P)
        .astype(BF16NP))
    w2p = np.ascontiguousarray(
        W2.reshape(4, 16, P, 16, P).transpose(0, 3, 2, 1, 4).reshape(4, 16, P, 16 * P)
        .astype(BF16NP))
    b1p = np.ascontiguousarray(np.asarray(inputs["b1"], f32).reshape(64, P).T)

    j = np.arange(P)[:, None, None]
    pp = np.arange(4)[None, :, None]
    cc = np.arange(512)[None, None, :]
    maskT = np.where(P * pp + j <= cc, 1.0, 0.0).astype(f32).reshape(P, 4 * 512)
    onesblk = np.ones((P, P), BF16NP)
    onesfblk = np.ones((P, P), f32)

    in_maps = []
    for c in range(N_CORES):
        sl = slice(CPC * c, CPC * (c + 1))
        bqs = (bq[sl] * scale).reshape(2, P).T
        bks = bk[sl].reshape(2, P).T
        in_maps.append({
            "embT": embT,
            "embres": np.ascontiguousarray(
                np.concatenate(
                    [embT[:, 256 * c:256 * (c + 1)],
                     embT[:, S + 256 * c:S + 256 * (c + 1)]], axis=1)
                .reshape(KE, P, TBLK).transpose(1, 0, 2).reshape(P, KE * TBLK)),
            "wq": _pack_w(Wq[:, sl] * scale).astype(BF16NP),
            "wk": _pack_w(Wk[:, sl]).astype(BF16NP),
            "wv": _pack_w(Wv[:, sl]).astype(BF16NP),
            "bqk": np.ascontiguousarray(np.concatenate([bqs, bks], axis=1)),
            "bvbc": np.ascontiguousarray(np.broadcast_to(bv[sl], (P, CPC))),
            "wfc": wfcp,
            "vecs": vecs,
            "w1": w1p,
            "b1": b1p,
            "w2": w2p,
            "maskT": maskT,
            "onesblk": onesblk,
            "onesfblk": onesfblk,
        })
    return in_maps


def kernel(**inputs) -> np.ndarray:
    nc = _get_program()
    in_maps = _prepare_in_maps(inputs)
    res = None
    last_err = None
    for attempt in range(3):
        try:
            res = run_bass_kernel_spmd(nc, in_maps, core_ids=list(range(N_CORES)))
            break
        except Exception as e:  # transient device/runtime hiccup: retry
            last_err = e
            import time as _time
            _time.sleep(3.0)
    if res is None:
        raise last_err
    out = np.empty((T, E), dtype=np.float32)
    for c in range(N_CORES):
        o = res.results[c]["outp"]          # [128, KE, 512] = [p, k, t]
        sl = o.transpose(1, 0, 2).reshape(E, TBLK)   # [E, 512]
        out[256 * c:256 * (c + 1)] = sl[:, 0:256].T
        out[S + 256 * c:S + 256 * (c + 1)] = sl[:, 256:].T
    return np.ascontiguousarray(out.reshape(B, S, E))


# revision 31
# speedup vs baseline: 1.0320x; 1.0320x over previous
"""Trainium2 Bass kernel for a single transformer decoder layer
(B=2, S=2048, E=2048, 16 heads, FFN 4x, causal attention, exact gelu,
two layernorms), distributed over 8 NeuronCores.

Sharding:
  - QKV + attention: tensor-parallel over heads (2 heads/core), zero comm.
  - One AllToAll per batch exchanges ctx slices ([head-slice, all tokens]
    -> [all heads, 256-token slice]); each core then runs the fc
    projection with the full Wfc plus LN1 + FFN (full W1/W2) + LN2 on its
    own 512-token slice (256 from each batch). Host concatenates the 8
    output slices.

Schedule: qkv(b0) -> attention(b0) -> fire a2a(b0) -> qkv(b1) ->
attention(b1) -> fire a2a(b1) -> fc -> LN1 -> FFN -> LN2. The first
all-to-all's rendezvous (which absorbs inter-core launch skew) overlaps
batch-1 qkv+attention instead of stalling the PE.

Everything on-chip stays transposed ([feature, token]) so biases and
layernorm gains are per-partition ops and no transposes are needed.
Matmuls run in float32r (~13-bit mantissa, bf16 speed at N>=512).
"""
import functools
import math

import ml_dtypes
import numpy as np

BF16NP = ml_dtypes.bfloat16

import concourse.bacc as bacc
import concourse.bass as bass
import concourse.mybir as mybir
import concourse.tile as tile
from concourse.bass_utils import run_bass_kernel_spmd

N_CORES = 8
P = 128
B, S, E = 2, 2048, 2048
T = B * S                   # 4096 tokens
NH, HD = 16, 128
FF = 4 * E                  # 8192
KE = E // P                 # 16 contraction chunks
CPC = 2 * HD                # 256 head-dim columns per core
TBLK = T // N_CORES         # 512 tokens per core after the all-to-all
EPS = 1e-5

F32 = mybir.dt.float32
F32R = mybir.dt.float32r
BF16 = mybir.dt.bfloat16

Identity = mybir.ActivationFunctionType.Identity
Copy = mybir.ActivationFunctionType.Copy
Exp = mybir.ActivationFunctionType.Exp
Gelu = mybir.ActivationFunctionType.Gelu
Sqrt = mybir.ActivationFunctionType.Sqrt
ADD = mybir.AluOpType.add
MULT = mybir.AluOpType.mult
SUB = mybir.AluOpType.subtract


def _ln_finish(nc, pool, psums, x_t, onesf, g_t, be_t, eps_t, out_t, tag,
               mu_ps=None, sq_ps=None, mu_sb=None, m2_sb=None,
               chunk_done=None, cs=slice(0, 512)):
    """Finish a layernorm over token columns `cs`. Stats arrive as raw-sum
    psums or as already-scaled SBUF tiles. Apply: two chunk-pair-wide DVE
    ops build (x - mu)*rstd, then the scalar engine applies the per-chunk
    gain/bias: out = in*g + be."""
    W = cs.stop - cs.start
    if mu_sb is None:
        mu_sb = pool.tile([1, 512], F32R, tag=f"{tag}_musb", bufs=1,
                          name=f"{tag}_musb")[:, 0:W]
        nc.scalar.activation(mu_sb, mu_ps[:, cs], Copy, scale=1.0 / E)
        m2_sb = pool.tile([1, 512], F32, tag=f"{tag}_m2sb", bufs=1,
                          name=f"{tag}_m2sb")[:, 0:W]
        nc.scalar.activation(m2_sb, sq_ps[:, cs], Copy, scale=1.0 / E)
    var = pool.tile([1, 512], F32, tag=f"{tag}_var", bufs=1,
                    name=f"{tag}_var")[:, 0:W]
    nc.vector.tensor_mul(var, mu_sb, mu_sb)
    nc.vector.tensor_sub(var, m2_sb, var)
    std = pool.tile([1, 512], F32, tag=f"{tag}_std", bufs=1,
                    name=f"{tag}_std")[:, 0:W]
    nc.scalar.activation(std, var, Sqrt, bias=eps_t[:])
    rstd = pool.tile([1, 512], F32, tag=f"{tag}_rstd", bufs=1,
                     name=f"{tag}_rstd")[:, 0:W]
    nc.vector.reciprocal_approx_fast(rstd, std)
    rstd_r = pool.tile([1, 512], F32R, tag=f"{tag}_rstdr", bufs=1,
                       name=f"{tag}_rstdr")[:, 0:W]
    musr_r = pool.tile([1, 512], F32R, tag=f"{tag}_musr", bufs=1,
                       name=f"{tag}_musr")[:, 0:W]
    with nc.allow_low_precision(reason="f32r ln broadcast operands"):
        nc.vector.tensor_copy(rstd_r, rstd)
        nc.vector.tensor_mul(musr_r, mu_sb, rstd)
    r_bc = psums.tile([P, 512], F32, tag=f"{tag}_rbc", bufs=1,
                      name=f"{tag}_rbc")[:, 0:W]
    nc.tensor.matmul(r_bc, onesf[0:1, :], rstd_r, start=True, stop=True)
    mr_bc = psums.tile([P, 512], F32, tag=f"{tag}_mrbc", bufs=1,
                       name=f"{tag}_mrbc")[:, 0:W]
    nc.tensor.matmul(mr_bc, onesf[0:1, :], musr_r, start=True, stop=True)
    for kp in range(0, KE, 2):
        m2v = pool.tile([P, 2, 512], F32, tag=f"{tag}_m1", bufs=2,
                        name=f"{tag}_m1")[:, :, 0:W]
        nc.vector.tensor_mul(
            m2v, x_t[:, kp:kp + 2, cs],
            r_bc.rearrange("p (j t) -> p j t", j=1).to_broadcast([P, 2, W]))
        nc.vector.tensor_sub(
            m2v, m2v,
            mr_bc.rearrange("p (j t) -> p j t", j=1).to_broadcast([P, 2, W]))
        for j in (0, 1):
            k = kp + j
            out_ap = out_t(k) if callable(out_t) else out_t[:, k, cs]
            nc.scalar.activation(out_ap, m2v[:, j, :], Identity,
                                 bias=be_t[:, k:k + 1],
                                 scale=g_t[:, k:k + 1])
            if chunk_done is not None:
                chunk_done(k, out_ap)


def _build_program():
    nc = bacc.Bacc("TRN2", target_bir_lowering=False, debug=False,
                   num_devices=N_CORES)

    # ---- per-core external inputs ----
    embT_d = nc.dram_tensor("embT", [E, T], BF16, kind="ExternalInput")
    embres_d = nc.dram_tensor("embres", [P, KE * TBLK], BF16, kind="ExternalInput")
    wq_d = nc.dram_tensor("wq", [P, KE * CPC], BF16, kind="ExternalInput")
    wk_d = nc.dram_tensor("wk", [P, KE * CPC], BF16, kind="ExternalInput")
    wv_d = nc.dram_tensor("wv", [P, KE * CPC], BF16, kind="ExternalInput")
    bqk_d = nc.dram_tensor("bqk", [P, 4], F32, kind="ExternalInput")  # bq|bk chunks
    bvbc_d = nc.dram_tensor("bvbc", [P, CPC], F32, kind="ExternalInput")
    wfc_d = nc.dram_tensor("wfc", [16, P, KE * P], BF16, kind="ExternalInput")
    vecs_d = nc.dram_tensor("vecs", [P, 6 * KE], F32, kind="ExternalInput")
    # vecs: [bfc | g1 | be1 | b2 | g2 | be2] each [P, KE]
    w1_d = nc.dram_tensor("w1", [64, P, KE * P], BF16, kind="ExternalInput")
    b1_d = nc.dram_tensor("b1", [P, 64], F32, kind="ExternalInput")
    w2_d = nc.dram_tensor("w2", [4, 16, P, 16 * P], BF16, kind="ExternalInput")
    mask_d = nc.dram_tensor("maskT", [P, 4 * 512], BF16, kind="ExternalInput")
    ones_d = nc.dram_tensor("onesblk", [P, P], BF16, kind="ExternalInput")
    onesf_d = nc.dram_tensor("onesfblk", [P, P], F32R, kind="ExternalInput")

    out_d = nc.dram_tensor("outp", [P, KE, TBLK], F32, kind="ExternalOutput")

    # ---- internal DRAM ----
    qT_d = nc.dram_tensor("qT_i", [CPC, T], BF16, kind="Internal")
    kT_d = nc.dram_tensor("kT_i", [CPC, T], BF16, kind="Internal")
    v_d = nc.dram_tensor("v_i", [T, CPC], F32R, kind="Internal")
    HB = TBLK // 2   # 256-token half-block
    a2a0in_d = nc.dram_tensor("a2a0in_i", [N_CORES, CPC, HB], BF16, kind="Internal")
    a2a0out_d = nc.dram_tensor("a2a0out_i", [N_CORES, CPC, HB], BF16, kind="Internal")
    a2a1in_d = nc.dram_tensor("a2a1in_i", [N_CORES, CPC, HB], BF16, kind="Internal")
    a2a1out_d = nc.dram_tensor("a2a1out_i", [N_CORES, CPC, HB], BF16, kind="Internal")

    with tile.TileContext(nc) as tc:
        with (
            tc.tile_pool(name="const", bufs=1) as cpool,
            tc.tile_pool(name="persist", bufs=1) as ppool,
        ):
            # the q-weight/embedding interleave below goes FIRST on the sync
            # queue so the first matmul can start ASAP; consts follow.
            ones = cpool.tile([P, P], BF16, name="ones")
            onesf = cpool.tile([P, P], F32R, name="onesf")
            mask_t = cpool.tile([P, 4, 512], BF16, name="mask_t")
            nc.scalar.dma_start(mask_t[:], mask_d[:].rearrange("p (f t) -> p f t", f=4))
            bqk_t = cpool.tile([P, 4], F32, name="bqk_t")
            nc.scalar.dma_start(bqk_t[:], bqk_d[:])
            bvbc_t = cpool.tile([P, CPC], F32, name="bvbc_t")
            nc.scalar.dma_start(bvbc_t[:], bvbc_d[:])
            vecs_t = cpool.tile([P, 6, KE], F32, name="vecs_t")
            nc.scalar.dma_start(vecs_t[:], vecs_d[:].rearrange("p (v k) -> p v k", v=6))
            b1_t = cpool.tile([P, 64], F32, name="b1_t")
            nc.scalar.dma_start(b1_t[:], b1_d[:])
            eps_t = cpool.tile([1, 1], F32, name="eps_t")
            nc.vector.memset(eps_t[:], EPS)

            bfc_t = vecs_t[:, 0, :]
            g1_t = vecs_t[:, 1, :]
            be1_t = vecs_t[:, 2, :]
            b2_t = vecs_t[:, 3, :]
            g2_t = vecs_t[:, 4, :]
            be2_t = vecs_t[:, 5, :]

            old_t = ppool.tile([P, KE, TBLK], BF16, name="old_t")   # LN1 output
            wfc_t = ppool.tile([P, 6, KE, P], BF16, name="wfc_lo")
            ctxL0 = ppool.tile([P, KE, HB], BF16, name="ctxL0")
            ctxL1 = ppool.tile([P, KE, HB], BF16, name="ctxL1")

            # ================= Phase Q: q/k/v projections =================
            with tc.tile_pool(name="qw", bufs=1) as qw:
                wq_t = qw.tile([P, KE, CPC], BF16, name="wq_t")
                wk_t = qw.tile([P, KE, CPC], BF16, name="wk_t")
                wv_t = qw.tile([P, KE, CPC], BF16, name="wv_t")

                def qkv_phase(b, qio, qps, tiles, e_pre=None, first=False):
                    if first:
                        e_pre = qio.tile([P, KE, 512], BF16, tag="emb", bufs=2,
                                         name="e_t")
                        wqv = wq_d[:].rearrange("p (k m) -> p k m", k=KE)
                        e0v = embT_d[:, 0:512].rearrange("(k p) t -> p k t", p=P)
                        for kc in range(0, KE, 4):
                            nc.sync.dma_start(wq_t[:, kc:kc + 4], wqv[:, kc:kc + 4])
                            nc.sync.dma_start(e_pre[:, kc:kc + 4], e0v[:, kc:kc + 4])
                        nc.sync.dma_start(
                            wk_t[:], wk_d[:].rearrange("p (k m) -> p k m", k=KE))
                        nc.sync.dma_start(
                            wv_t[:], wv_d[:].rearrange("p (k m) -> p k m", k=KE))
                        nc.sync.dma_start(ones[:], ones_d[:])
                        nc.sync.dma_start(onesf[:], onesf_d[:])
                        # prefetch the fc weights on the scalar engine's DMA
                        # queue so they stream during Q/A instead of stalling F
                        for nb in range(6):
                            nc.scalar.dma_start(
                                wfc_t[:, nb],
                                wfc_d.ap()[nb].rearrange("p (k m) -> p k m", k=KE))

                    for tbl in range(4):
                        tb = 4 * b + tbl
                        if tbl == 0 and e_pre is not None:
                            e_t = e_pre
                        else:
                            e_t = qio.tile([P, KE, 512], BF16, tag="emb", bufs=2,
                                           name="e_t")
                            nc.sync.dma_start(
                                e_t[:],
                                embT_d[:, tb * 512:(tb + 1) * 512]
                                .rearrange("(k p) t -> p k t", p=P),
                            )
                        for wi, (wt, dst) in enumerate(((wq_t, qT_d), (wk_t, kT_d))):
                            for hc in range(2):
                                pqk = qps.tile([P, 512], F32, tag="pqk", bufs=3,
                                               name="pqk")
                                for k in range(KE):
                                    nc.tensor.matmul(
                                        pqk[:], wt[:, k, hc * P:(hc + 1) * P],
                                        e_t[:, k, :],
                                        start=(k == 0), stop=(k == KE - 1),
                                    )
                                st = qio.tile([P, 512], BF16, tag="qkst", bufs=2,
                                              name="st")
                                nc.scalar.activation(
                                    st[:], pqk[:], Identity,
                                    bias=bqk_t[:, 2 * wi + hc:2 * wi + hc + 1])
                                nc.sync.dma_start(
                                    dst.ap()[hc * P:(hc + 1) * P,
                                             tb * 512:(tb + 1) * 512],
                                    st[:])
                        for tt in range(4):
                            pv = qps.tile([P, CPC], F32, tag="pv", bufs=3, name="pv")
                            for k in range(KE):
                                nc.tensor.matmul(
                                    pv[:], e_t[:, k, tt * P:(tt + 1) * P], wv_t[:, k, :],
                                    start=(k == 0), stop=(k == KE - 1),
                                )
                            vst = qio.tile([P, CPC], F32R, tag="vst", bufs=2, name="vst")
                            with nc.allow_low_precision(reason="f32r v store"):
                                nc.vector.tensor_add(vst[:], pv[:], bvbc_t[:])
                            nc.sync.dma_start(
                                v_d.ap()[tb * 512 + tt * P: tb * 512 + (tt + 1) * P, :],
                                vst[:])
                        # emit this block's attention loads NOW: their DRAM
                        # deps cover only the stores emitted so far, so each
                        # slice streams in as soon as this block's stores
                        # land instead of waiting for the whole phase
                        ts = slice(b * S + tbl * 512, b * S + (tbl + 1) * 512)
                        ls = slice(tbl * 512, (tbl + 1) * 512)
                        for hc in range(2):
                            q_t, k_t, v_t = tiles[hc]
                            nc.gpsimd.dma_start(
                                q_t[:, ls], qT_d.ap()[hc * P:(hc + 1) * P, ts])
                            nc.gpsimd.dma_start(
                                k_t[:, ls], kT_d.ap()[hc * P:(hc + 1) * P, ts])
                            nc.gpsimd.dma_start(
                                v_t[:, 4 * tbl:4 * tbl + 4, :],
                                v_d.ap()[ts, hc * P:(hc + 1) * P]
                                .rearrange("(j p) d -> p j d", p=P),
                            )

                # ============ Phase A: causal attention (per batch) ========
                # scoresT/ctxT per head, all transposed; softmax denom via
                # ones-matmul; sc emission pipelined 2 deep; per-q-tile
                # normalization deferred one q-tile so the PE never waits on
                # the DVE chain (except the last tile of the batch, finalized
                # immediately so the all-to-all fires as early as possible).
                def attn_phase(b, tiles, asc, aps):
                    a2ain = a2a0in_d if b == 0 else a2a1in_d
                    pending = [None]

                    def finalize(st):
                        hc, qt, ctx_ps, l_ps = st
                        l_sb = asc.tile([1, 512], F32, tag="lsb", bufs=2,
                                        name="l_sb")
                        nc.vector.tensor_copy(l_sb[:], l_ps[:])
                        r_sb = asc.tile([1, 512], F32, tag="rsb", bufs=2,
                                        name="r_sb")
                        nc.vector.reciprocal_approx_fast(r_sb[:], l_sb[:])
                        r_r = asc.tile([1, 512], BF16, tag="rr", bufs=2, name="r_r")
                        with nc.allow_low_precision(reason="bf16 recip bcast"):
                            nc.vector.tensor_copy(r_r[:], r_sb[:])
                        rbc_ps = aps.tile([P, 512], F32, tag="sc", bufs=2,
                                          name="rbc_ps")
                        nc.tensor.matmul(rbc_ps[:], ones[0:1, :], r_r[:],
                                         start=True, stop=True)
                        ctx_sb = asc.tile([P, 512], F32, tag="ctxsb", bufs=2,
                                          name="ctx_sb")
                        # on the vector queue (not scalar) so the next tile's
                        # Exp is never queued behind this copy
                        nc.vector.tensor_copy(ctx_sb[:], ctx_ps[:])
                        ctx_f = asc.tile([P, 512], BF16, tag="ctxf", bufs=2,
                                         name="ctx_f")
                        with nc.allow_low_precision(reason="bf16 ctx for a2a"):
                            nc.vector.tensor_mul(ctx_f[:], ctx_sb[:], rbc_ps[:])
                        nc.sync.dma_start(
                            a2ain.ap()[2 * qt, hc * P:(hc + 1) * P, :],
                            ctx_f[:, 0:HB])
                        nc.sync.dma_start(
                            a2ain.ap()[2 * qt + 1, hc * P:(hc + 1) * P, :],
                            ctx_f[:, HB:])

                    for hc in range(2):
                        q_t, k_t, v_t = tiles[hc]
                        for qt in range(4):
                            nkb = 4 * qt + 4
                            ctx_ps = aps.tile([P, 512], F32, tag="ctx", bufs=2,
                                              name="ctx_ps")
                            l_full = aps.tile([P, 512], F32, tag="lr", bufs=2,
                                              name="l_full")
                            l_ps = l_full[0:1, :]
                            ex_pairs = [None] * nkb
                            ex_tiles = [None] * nkb
                            sc_cur = [None]

                            def emit_sc(kb, qt=qt, k_t=k_t, q_t=q_t,
                                        ex_tiles=ex_tiles, sc_cur=sc_cur,
                                        ex_pairs=ex_pairs):
                                # kb-blocks are processed in pairs sharing one
                                # 2-bank psum tile and a single wide Exp.
                                # Causal mask is a 0/1 DVE multiply; the
                                # softmax denominator accumulates on the PE.
                                half = kb % 2
                                if half == 0:
                                    sc_cur[0] = aps.tile([P, 2, 512], F32,
                                                         tag="sc", bufs=2,
                                                         name="sc_ps")
                                sc_ps = sc_cur[0]
                                # causally-dead q columns of diagonal blocks
                                # are never written (stale psum is finite; the
                                # 0/1 mask multiply zeroes exp of it)
                                d = max(0, (kb - 4 * qt)) * P
                                nc.tensor.matmul(
                                    sc_ps[:, half, d:],
                                    k_t[:, kb * P:(kb + 1) * P],
                                    q_t[:, qt * 512 + d:(qt + 1) * 512],
                                    start=True, stop=True)
                                if half == 1:
                                    ex = asc.tile([P, 2, 512], F32R, tag="ex",
                                                  bufs=3, name="ex")
                                    ds = [max(0, kb - 1 - 4 * qt) * P,
                                          max(0, kb - 4 * qt) * P]
                                    any_diag = kb >= 4 * qt
                                    # per-half exp always: the first ctx
                                    # matmul only waits on half 0, and diag
                                    # blocks never read unwritten psum
                                    for hh in (0, 1):
                                        nc.scalar.activation(
                                            ex[:, hh, ds[hh]:],
                                            sc_ps[:, hh, ds[hh]:], Exp)
                                    with nc.allow_low_precision(reason="attn mask"):
                                        for hh in (0, 1):
                                            kbb = kb - 1 + hh
                                            dd = ds[hh]
                                            if kbb >= 4 * qt:
                                                # triangular mask on the
                                                # diagonal 128-col block only
                                                nc.vector.tensor_mul(
                                                    ex[:, hh, dd:dd + P],
                                                    ex[:, hh, dd:dd + P],
                                                    mask_t[:, kbb - 4 * qt,
                                                           dd:dd + P])
                                    ex_pairs[kb] = (ex, any_diag, ds)
                                    ex_tiles[kb - 1] = ex[:, 0, :]
                                    ex_tiles[kb] = ex[:, 1, :]

                            for w in range(min(4, nkb)):
                                emit_sc(w)
                            for kb in range(nkb):
                                if kb + 4 < nkb:
                                    emit_sc(kb + 4)
                                d = max(0, (kb - 4 * qt)) * P
                                nc.tensor.matmul(ctx_ps[:, d:], v_t[:, kb, :],
                                                 ex_tiles[kb][:, d:],
                                                 start=(kb == 0),
                                                 stop=(kb == nkb - 1))
                                ex_tiles[kb] = None
                                if kb % 2 == 1:
                                    # softmax denominator rides the PE,
                                    # clipped to the live range per half
                                    ex, any_diag, ds = ex_pairs[kb]
                                    last = kb == nkb - 1
                                    nc.tensor.matmul(
                                        l_ps[:, ds[0]:], onesf[:, 0:1],
                                        ex[:, 0, ds[0]:],
                                        start=(kb == 1), stop=False)
                                    nc.tensor.matmul(
                                        l_ps[:, ds[1]:], onesf[:, 0:1],
                                        ex[:, 1, ds[1]:],
                                        start=False, stop=last)
                                    ex_pairs[kb] = None
                            if pending[0] is not None:
                                finalize(pending[0])
                            pending[0] = (hc, qt, ctx_ps, l_ps)
                    # last tile: finalize immediately so the all-to-all for
                    # this batch fires as early as possible
                    finalize(pending[0])

                with (
                    tc.tile_pool(name="aio", bufs=1) as aio,
                    tc.tile_pool(name="asc", bufs=1) as asc,
                ):
                    def alloc_tiles():
                        out = {}
                        for hc in range(2):
                            out[hc] = (
                                aio.tile([P, S], BF16, tag="q", bufs=2,
                                         name="q_t"),
                                aio.tile([P, S], BF16, tag="k", bufs=2,
                                         name="k_t"),
                                aio.tile([P, 16, P], F32R, tag="v", bufs=2,
                                         name="v_t"),
                            )
                        return out

                    tiles0 = alloc_tiles()
                    with (
                        tc.tile_pool(name="qio0", bufs=2) as qio0,
                        tc.tile_pool(name="qps0", bufs=1, space="PSUM") as qps0,
                    ):
                        qkv_phase(0, qio0, qps0, tiles0, first=True)
                    with tc.tile_pool(name="qio1", bufs=2) as qio1:
                        with tc.tile_pool(name="aps0", bufs=1,
                                          space="PSUM") as aps0:
                            attn_phase(0, tiles0, asc, aps0)
                        # prefetch batch-1's first embedding block on the
                        # gpsimd queue (idle after the attention loads) so
                        # qkv(b1) starts the moment attention(b0) ends
                        e_pre1 = qio1.tile([P, KE, 512], BF16, tag="emb",
                                           bufs=2, name="e_t")
                        nc.gpsimd.dma_start(
                            e_pre1[:],
                            embT_d[:, 4 * 512:5 * 512]
                            .rearrange("(k p) t -> p k t", p=P))
                        # batch-0 ctx fully written -> exchange it while
                        # batch-1 qkv + attention run.
                        nc.gpsimd.collective_compute(
                            "AllToAll", mybir.AluOpType.bypass,
                            replica_groups=[list(range(N_CORES))],
                            ins=[a2a0in_d.ap()], outs=[a2a0out_d.ap()],
                        )
                        tiles1 = alloc_tiles()
                        with tc.tile_pool(name="qps1", bufs=1,
                                          space="PSUM") as qps1:
                            qkv_phase(1, qio1, qps1, tiles1, e_pre=e_pre1)
                    with tc.tile_pool(name="aps1", bufs=1,
                                      space="PSUM") as aps1:
                        attn_phase(1, tiles1, asc, aps1)
                # ctxL0 load sits AFTER batch-1's q/k/v loads on the gpsimd
                # queue so attention never queues behind a collective wait.
                nc.gpsimd.dma_start(
                    ctxL0[:],
                    a2a0out_d.ap().rearrange("r (c p) t -> p (r c) t", p=P))
                nc.gpsimd.collective_compute(
                    "AllToAll", mybir.AluOpType.bypass,
                    replica_groups=[list(range(N_CORES))],
                    ins=[a2a1in_d.ap()], outs=[a2a1out_d.ap()],
                )
                nc.gpsimd.dma_start(
                    ctxL1[:],
                    a2a1out_d.ap().rearrange("r (c p) t -> p (r c) t", p=P))

            # ====== Phase F: fc with full Wfc + residual + LN1 stats =======
            # nwp (FFN weight/hidden tiles) opens BEFORE fio so its tiles
            # don't alias x_t: the first w1 loads then stream during fc
            # instead of waiting for LN1 to release x_t's memory.
            with tc.tile_pool(name="nwp", bufs=1) as nwp:
                y_sb = nwp.tile([P, KE, TBLK], F32, name="y_sb")  # FFN accum
                with (
                    tc.tile_pool(name="fio", bufs=1) as fio,
                    tc.tile_pool(name="fps", bufs=1, space="PSUM") as fps,
                ):
                    x_t = fio.tile([P, KE, TBLK], BF16, name="x_t")
                    mu_ps = fps.tile([1, 512], F32, tag="ln1_mu", bufs=1,
                                     name="ln1_mu")
                    sq_ps = fps.tile([1, 512], F32, tag="ln1_sq", bufs=1,
                                     name="ln1_sq")

                    def fc_stats(nb, cs):
                        nc.tensor.matmul(mu_ps[:, cs], ones[:, 0:1],
                                         x_t[:, nb, cs],
                                         start=(nb == 0), stop=(nb == 15))
                        sqk = fio.tile([P, HB], BF16, tag="sqk", bufs=3,
                                       name="sqk")
                        with nc.allow_low_precision(reason="bf16 ln1 squares"):
                            nc.vector.tensor_mul(sqk[:], x_t[:, nb, cs],
                                                 x_t[:, nb, cs])
                        nc.tensor.matmul(sq_ps[:, cs], ones[:, 0:1], sqk[:],
                                         start=(nb == 0), stop=(nb == 15))

                    wfc_hi = {}
                    for h in range(2):
                        cs = slice(h * HB, (h + 1) * HB)
                        ctxh = ctxL0 if h == 0 else ctxL1
                        for nb in range(16):
                            if nb < 6:
                                wnb = wfc_t[:, nb]
                            elif nb < 10:
                                # cached across both column halves
                                if h == 0:
                                    whi = fio.tile([P, KE, P], BF16,
                                                   tag="wfchi", bufs=4,
                                                   name="wfc_hi")
                                    nc.sync.dma_start(
                                        whi[:],
                                        wfc_d.ap()[nb].rearrange(
                                            "p (k m) -> p k m", k=KE))
                                    wfc_hi[nb] = whi
                                wnb = wfc_hi[nb][:]
                            else:
                                # re-streamed per half (SBUF pressure)
                                whi = fio.tile([P, KE, P], BF16, tag="wfcs",
                                               bufs=2, name="wfc_s")
                                nc.sync.dma_start(
                                    whi[:],
                                    wfc_d.ap()[nb].rearrange(
                                        "p (k m) -> p k m", k=KE))
                                wnb = whi[:]
                            embres_t = fio.tile([P, HB], BF16, tag="embres",
                                                bufs=3, name="embres_t")
                            nc.sync.dma_start(
                                embres_t[:],
                                embres_d[:, nb * TBLK + h * HB:
                                         nb * TBLK + (h + 1) * HB])
                            pfc = fps.tile([P, HB], F32, tag="pfc", bufs=3,
                                           name="pfc")
                            for k in range(KE):
                                nc.tensor.matmul(pfc[:], wnb[:, k, :],
                                                 ctxh[:, k, :],
                                                 start=(k == 0),
                                                 stop=(k == KE - 1))
                            with nc.allow_low_precision(reason="bf16 ln1 input"):
                                nc.vector.scalar_tensor_tensor(
                                    x_t[:, nb, cs], pfc[:], bfc_t[:, nb:nb + 1],
                                    embres_t[:], ADD, ADD)
                            if nb > 0:
                                fc_stats(nb - 1, cs)
                        fc_stats(15, cs)
                        _ln_finish(nc, fio, fps, x_t, onesf, g1_t, be1_t,
                                   eps_t, old_t, "ln1", mu_ps=mu_ps,
                                   sq_ps=sq_ps, cs=cs)

                # ======== Phase N: FFN with LN2 input + stats fused ========
                with tc.tile_pool(name="l2", bufs=1) as l2p:
                    x2_t = l2p.tile([P, KE, TBLK], F32R, name="x2_t")
                    mu2_sb = l2p.tile([1, 512], F32R, name="mu2_sb")
                    m22_sb = l2p.tile([1, 512], F32, name="m22_sb")
                    with tc.tile_pool(name="nps", bufs=1, space="PSUM") as nps:
                        mu2_ps = nps.tile([1, 512], F32, tag="ln2_mu", bufs=1,
                                          name="ln2_mu")
                        sq2_ps = nps.tile([1, 512], F32, tag="ln2_sq", bufs=1,
                                          name="ln2_sq")

                        class _HpsShim:
                            """_ln_finish psum allocator that reuses the idle
                            hps tag so ln2 can run inside the nps scope."""
                            def tile(self, shape, dtype, tag=None, bufs=1,
                                     name=None):
                                return nps.tile(shape, dtype, tag="hps",
                                                bufs=3, name=name)
                        hps_shim = _HpsShim()

                        def y_accum(hbg, nb, c2, h_t):
                            w2_t = nwp.tile([P, 16, P], BF16, tag="w2", bufs=2,
                                            name="w2_t")
                            nc.sync.dma_start(
                                w2_t[:],
                                w2_d.ap()[hbg, nb].rearrange(
                                    "p (l m) -> p l m", l=16))
                            yps = nps.tile([P, 512], F32, tag="yps", bufs=3,
                                           name="yps")[:, 0:c2.stop - c2.start]
                            for hl in range(16):
                                nc.tensor.matmul(yps, w2_t[:, hl, :],
                                                 h_t[:, hl, c2],
                                                 start=(hl == 0),
                                                 stop=(hl == 15))
                            return yps

                        for hbg in range(4):
                            h_t = nwp.tile([P, 16, TBLK], BF16, tag="h",
                                           bufs=1, name="h_t")
                            if hbg == 0:
                                # bridge: ALL h0-column halves of the first 3
                                # blocks run while ln1-h1 is still applying,
                                # then their h1 halves. Per-half gelu.
                                pre_w1 = []
                                pre_hps = []
                                for hl in range(3):
                                    w1_t = nwp.tile([P, KE, P], BF16,
                                                    tag="w1", bufs=3,
                                                    name="w1_t")
                                    nc.sync.dma_start(
                                        w1_t[:],
                                        w1_d.ap()[hl].rearrange(
                                            "p (k m) -> p k m", k=KE))
                                    pre_w1.append(w1_t)
                                    pre_hps.append(nps.tile(
                                        [P, 512], F32, tag="hps", bufs=3,
                                        name="hps"))
                                for h2 in range(2):
                                    c2 = slice(h2 * HB, (h2 + 1) * HB)
                                    for hl in range(3):
                                        for k in range(KE):
                                            nc.tensor.matmul(
                                                pre_hps[hl][:, c2],
                                                pre_w1[hl][:, k, :],
                                                old_t[:, k, c2],
                                                start=(k == 0),
                                                stop=(k == KE - 1))
                                        nc.scalar.activation(
                                            h_t[:, hl, c2],
                                            pre_hps[hl][:, c2], Gelu,
                                            bias=b1_t[:, hl:hl + 1])
                            for hl in range(3 if hbg == 0 else 0, 16):
                                hb = hbg * 16 + hl
                                w1_t = nwp.tile([P, KE, P], BF16, tag="w1",
                                                bufs=3, name="w1_t")
                                nc.sync.dma_start(
                                    w1_t[:],
                                    w1_d.ap()[hb].rearrange("p (k m) -> p k m",
                                                            k=KE))
                                hps = nps.tile([P, 512], F32, tag="hps",
                                               bufs=3, name="hps")
                                for k in range(KE):
                                    nc.tensor.matmul(hps[:], w1_t[:, k, :],
                                                     old_t[:, k, :],
                                                     start=(k == 0),
                                                     stop=(k == KE - 1))
                                nc.scalar.activation(h_t[:, hl, :], hps[:],
                                                     Gelu,
                                                     bias=b1_t[:, hb:hb + 1])
                            if hbg < 3:
                                for nb in range(16):
                                    yps = y_accum(hbg, nb, slice(0, 512), h_t)
                                    if hbg == 0:
                                        nc.vector.tensor_copy(y_sb[:, nb, :],
                                                              yps)
                                    else:
                                        nc.vector.tensor_add(y_sb[:, nb, :],
                                                             y_sb[:, nb, :],
                                                             yps)
                            else:
                                # last group: split by column half so ln2 on
                                # half 0 overlaps the half-1 matmuls; its
                                # emission is deferred a few nb groups so the
                                # PE never waits on the ln2-h0 DVE chain
                                ln_pending = [None]
                                for h2 in range(2):
                                    c2 = slice(h2 * HB, (h2 + 1) * HB)
                                    for nb in range(16):
                                        if (h2 == 1 and nb == 4
                                                and ln_pending[0] is not None):
                                            ln_pending[0]()
                                            ln_pending[0] = None
                                        yps = y_accum(hbg, nb, c2, h_t)
                                        nc.vector.tensor_add(y_sb[:, nb, c2],
                                                             y_sb[:, nb, c2],
                                                             yps)
                                        nc.vector.scalar_tensor_tensor(
                                            x2_t[:, nb, c2], y_sb[:, nb, c2],
                                            b2_t[:, nb:nb + 1],
                                            old_t[:, nb, c2], ADD, ADD)
                                        sq2 = nwp.tile([P, 512], F32R,
                                                       tag="sq2", bufs=1,
                                                       name="sq2")[:, 0:HB]
                                        nc.vector.tensor_mul(
                                            sq2, x2_t[:, nb, c2],
                                            x2_t[:, nb, c2])
                                        nc.tensor.matmul(mu2_ps[:, c2],
                                                         onesf[:, 0:1],
                                                         x2_t[:, nb, c2],
                                                         start=(nb == 0),
                                                         stop=(nb == 15))
                                        nc.tensor.matmul(sq2_ps[:, c2],
                                                         onesf[:, 0:1], sq2,
                                                         start=(nb == 0),
                                                         stop=(nb == 15))
                                    nc.scalar.activation(mu2_sb[:, c2],
                                                         mu2_ps[:, c2], Copy,
                                                         scale=1.0 / E)
                                    nc.scalar.activation(m22_sb[:, c2],
                                                         sq2_ps[:, c2], Copy,
                                                         scale=1.0 / E)

                                    def fin(c2=c2):
                                        _ln_finish(
                                            nc, l2p, hps_shim, x2_t, onesf,
                                            g2_t, be2_t, eps_t,
                                            lambda k: l2p.tile(
                                                [P, 512], F32, tag="osb",
                                                bufs=2, name="out_sb")[:, 0:HB],
                                            "ln2", mu_sb=mu2_sb[:, c2],
                                            m2_sb=m22_sb[:, c2],
                                            chunk_done=lambda k, ap, c2=c2:
                                                nc.sync.dma_start(
                                                    out_d.ap()[:, k, c2], ap),
                                            cs=c2)
                                    if h2 == 0:
                                        ln_pending[0] = fin
                                    else:
                                        fin()

    nc.compile()
    return nc


@functools.lru_cache(maxsize=1)
def _get_program():
    return _build_program()


def _pack_w(w):
    """[E_rows, M] -> [128, (E_rows/128)*M] with [p, k, m] layout."""
    e, m = w.shape
    return np.ascontiguousarray(
        w.reshape(e // P, P, m).transpose(1, 0, 2).reshape(P, -1))


def _pack_vec(v):
    """[n*128] -> [128, n] per-partition chunks."""
    return np.ascontiguousarray(v.reshape(-1, P).T)


def _prepare_in_maps(inputs):
    f32 = np.float32
    emb = np.asarray(inputs["embeddings"], f32).reshape(T, E)
    embT = np.ascontiguousarray(emb.T.astype(BF16NP))
    scale = 1.0 / math.sqrt(HD)

    Wq = np.asarray(inputs["Wq"], f32)
    Wk = np.asarray(inputs["Wk"], f32)
    Wv = np.asarray(inputs["Wv"], f32)
    bq = np.asarray(inputs["bq"], f32)
    bk = np.asarray(inputs["bk"], f32)
    bv = np.asarray(inputs["bv"], f32)
    Wfc = np.asarray(inputs["Wfc"], f32)
    W1 = np.asarray(inputs["W1"], f32)
    W2 = np.asarray(inputs["W2"], f32)

    vecs = np.concatenate([
        _pack_vec(np.asarray(inputs[n], f32))
        for n in ("bfc", "g1", "be1", "b2", "g2", "be2")
    ], axis=1)  # [128, 6*KE]

    wfcp = np.ascontiguousarray(
        Wfc.reshape(KE, P, 16, P).transpose(2, 1, 0, 3).reshape(16, P, KE * P)
        .astype(BF16NP))
    w1p = np.ascontiguousarray(
        W1.reshape(KE, P, 64, P).transpose(2, 1, 0, 3).reshape(64, P, KE * P)
        .astype(BF16NP))
    w2p = np.ascontiguousarray(
        W2.reshape(4, 16, P, 16, P).transpose(0, 3, 2, 1, 4).reshape(4, 16, P, 16 * P)
        .astype(BF16NP))
    b1p = np.ascontiguousarray(np.asarray(inputs["b1"], f32).reshape(64, P).T)

    j = np.arange(P)[:, None, None]
    pp = np.arange(4)[None, :, None]
    cc = np.arange(512)[None, None, :]
    maskT = np.where(P * pp + j <= cc, 1.0, 0.0).astype(BF16NP).reshape(P, 4 * 512)
    onesblk = np.ones((P, P), BF16NP)
    onesfblk = np.ones((P, P), f32)

    in_maps = []
    for c in range(N_CORES):
        sl = slice(CPC * c, CPC * (c + 1))
        bqs = (bq[sl] * scale).reshape(2, P).T
        bks = bk[sl].reshape(2, P).T
        in_maps.append({
            "embT": embT,
            "embres": np.ascontiguousarray(
                np.concatenate(
                    [embT[:, 256 * c:256 * (c + 1)],
                     embT[:, S + 256 * c:S + 256 * (c + 1)]], axis=1)
                .reshape(KE, P, TBLK).transpose(1, 0, 2).reshape(P, KE * TBLK)),
            "wq": _pack_w(Wq[:, sl] * scale).astype(BF16NP),
            "wk": _pack_w(Wk[:, sl]).astype(BF16NP),
            "wv": _pack_w(Wv[:, sl]).astype(BF16NP),
            "bqk": np.ascontiguousarray(np.concatenate([bqs, bks], axis=1)),
            "bvbc": np.ascontiguousarray(np.broadcast_to(bv[sl], (P, CPC))),
            "wfc": wfcp,
            "vecs": vecs,
            "w1": w1p,
            "b1": b1p,
            "w2": w2p,
            "maskT": maskT,
            "onesblk": onesblk,
            "onesfblk": onesfblk,
        })
    return in_maps


def kernel(**inputs) -> np.ndarray:
    nc = _get_program()
    in_maps = _prepare_in_maps(inputs)
    res = None
    last_err = None
    for attempt in range(3):
        try:
            res = run_bass_kernel_spmd(nc, in_maps, core_ids=list(range(N_CORES)))
            break
        except Exception as e:  # transient device/runtime hiccup: retry
            last_err = e
            import time as _time
            _time.sleep(3.0)
    if res is None:
        raise last_err
    out = np.empty((T, E), dtype=np.float32)
    for c in range(N_CORES):
        o = res.results[c]["outp"]          # [128, KE, 512] = [p, k, t]
        sl = o.transpose(1, 0, 2).reshape(E, TBLK)   # [E, 512]
        out[256 * c:256 * (c + 1)] = sl[:, 0:256].T
        out[S + 256 * c:S + 256 * (c + 1)] = sl[:, 256:].T
    return np.ascontiguousarray(out.reshape(B, S, E))


# revision 35
# speedup vs baseline: 1.0380x; 1.0059x over previous
"""Trainium2 Bass kernel for a single transformer decoder layer
(B=2, S=2048, E=2048, 16 heads, FFN 4x, causal attention, exact gelu,
two layernorms), distributed over 8 NeuronCores.

Sharding:
  - QKV + attention: tensor-parallel over heads (2 heads/core), zero comm.
  - One AllToAll per batch exchanges ctx slices ([head-slice, all tokens]
    -> [all heads, 256-token slice]); each core then runs the fc
    projection with the full Wfc plus LN1 + FFN (full W1/W2) + LN2 on its
    own 512-token slice (256 from each batch). Host concatenates the 8
    output slices.

Schedule: qkv(b0) -> attention(b0) -> fire a2a(b0) -> qkv(b1) ->
attention(b1) -> fire a2a(b1) -> fc -> LN1 -> FFN -> LN2. The first
all-to-all's rendezvous (which absorbs inter-core launch skew) overlaps
batch-1 qkv+attention instead of stalling the PE.

Everything on-chip stays transposed ([feature, token]) so biases and
layernorm gains are per-partition ops and no transposes are needed.
Matmuls run in float32r (~13-bit mantissa, bf16 speed at N>=512).
"""
import functools
import math

import ml_dtypes
import numpy as np

BF16NP = ml_dtypes.bfloat16

import concourse.bacc as bacc
import concourse.bass as bass
import concourse.mybir as mybir
import concourse.tile as tile
from concourse.bass_utils import run_bass_kernel_spmd

N_CORES = 8
P = 128
B, S, E = 2, 2048, 2048
T = B * S                   # 4096 tokens
NH, HD = 16, 128
FF = 4 * E                  # 8192
KE = E // P                 # 16 contraction chunks
CPC = 2 * HD                # 256 head-dim columns per core
TBLK = T // N_CORES         # 512 tokens per core after the all-to-all
EPS = 1e-5

F32 = mybir.dt.float32
F32R = mybir.dt.float32r
BF16 = mybir.dt.bfloat16

Identity = mybir.ActivationFunctionType.Identity
Copy = mybir.ActivationFunctionType.Copy
Exp = mybir.ActivationFunctionType.Exp
Gelu = mybir.ActivationFunctionType.Gelu
Sqrt = mybir.ActivationFunctionType.Sqrt
ADD = mybir.AluOpType.add
MULT = mybir.AluOpType.mult
SUB = mybir.AluOpType.subtract


def _ln_finish(nc, pool, psums, x_t, onesf, g_t, be_t, eps_t, out_t, tag,
               mu_ps=None, sq_ps=None, mu_sb=None, m2_sb=None,
               chunk_done=None, cs=slice(0, 512)):
    """Finish a layernorm over token columns `cs`. Stats arrive as raw-sum
    psums or as already-scaled SBUF tiles. Apply: two chunk-pair-wide DVE
    ops build (x - mu)*rstd, then the scalar engine applies the per-chunk
    gain/bias: out = in*g + be."""
    W = cs.stop - cs.start
    if mu_sb is None:
        mu_sb = pool.tile([1, 512], F32R, tag=f"{tag}_musb", bufs=1,
                          name=f"{tag}_musb")[:, 0:W]
        nc.scalar.activation(mu_sb, mu_ps[:, cs], Copy, scale=1.0 / E)
        m2_sb = pool.tile([1, 512], F32, tag=f"{tag}_m2sb", bufs=1,
                          name=f"{tag}_m2sb")[:, 0:W]
        nc.scalar.activation(m2_sb, sq_ps[:, cs], Copy, scale=1.0 / E)
    var = pool.tile([1, 512], F32, tag=f"{tag}_var", bufs=1,
                    name=f"{tag}_var")[:, 0:W]
    nc.vector.tensor_mul(var, mu_sb, mu_sb)
    nc.vector.tensor_sub(var, m2_sb, var)
    std = pool.tile([1, 512], F32, tag=f"{tag}_std", bufs=1,
                    name=f"{tag}_std")[:, 0:W]
    nc.scalar.activation(std, var, Sqrt, bias=eps_t[:])
    rstd = pool.tile([1, 512], F32, tag=f"{tag}_rstd", bufs=1,
                     name=f"{tag}_rstd")[:, 0:W]
    nc.vector.reciprocal_approx_fast(rstd, std)
    rstd_r = pool.tile([1, 512], F32R, tag=f"{tag}_rstdr", bufs=1,
                       name=f"{tag}_rstdr")[:, 0:W]
    musr_r = pool.tile([1, 512], F32R, tag=f"{tag}_musr", bufs=1,
                       name=f"{tag}_musr")[:, 0:W]
    with nc.allow_low_precision(reason="f32r ln broadcast operands"):
        nc.vector.tensor_copy(rstd_r, rstd)
        nc.vector.tensor_mul(musr_r, mu_sb, rstd)
    r_bc = psums.tile([P, 512], F32, tag=f"{tag}_rbc", bufs=1,
                      name=f"{tag}_rbc")[:, 0:W]
    nc.tensor.matmul(r_bc, onesf[0:1, :], rstd_r, start=True, stop=True)
    mr_bc = psums.tile([P, 512], F32, tag=f"{tag}_mrbc", bufs=1,
                       name=f"{tag}_mrbc")[:, 0:W]
    nc.tensor.matmul(mr_bc, onesf[0:1, :], musr_r, start=True, stop=True)
    for kp in range(0, KE, 2):
        m2v = pool.tile([P, 2, 512], F32, tag=f"{tag}_m1", bufs=2,
                        name=f"{tag}_m1")[:, :, 0:W]
        nc.vector.tensor_mul(
            m2v, x_t[:, kp:kp + 2, cs],
            r_bc.rearrange("p (j t) -> p j t", j=1).to_broadcast([P, 2, W]))
        nc.vector.tensor_sub(
            m2v, m2v,
            mr_bc.rearrange("p (j t) -> p j t", j=1).to_broadcast([P, 2, W]))
        for j in (0, 1):
            k = kp + j
            out_ap = out_t(k) if callable(out_t) else out_t[:, k, cs]
            nc.scalar.activation(out_ap, m2v[:, j, :], Identity,
                                 bias=be_t[:, k:k + 1],
                                 scale=g_t[:, k:k + 1])
            if chunk_done is not None:
                chunk_done(k, out_ap)


def _build_program():
    nc = bacc.Bacc("TRN2", target_bir_lowering=False, debug=False,
                   num_devices=N_CORES)

    # ---- per-core external inputs ----
    embT_d = nc.dram_tensor("embT", [E, T], BF16, kind="ExternalInput")
    embres_d = nc.dram_tensor("embres", [P, KE * TBLK], BF16, kind="ExternalInput")
    wq_d = nc.dram_tensor("wq", [P, KE * CPC], BF16, kind="ExternalInput")
    wk_d = nc.dram_tensor("wk", [P, KE * CPC], BF16, kind="ExternalInput")
    wv_d = nc.dram_tensor("wv", [P, KE * CPC], BF16, kind="ExternalInput")
    bqk_d = nc.dram_tensor("bqk", [P, 4], F32, kind="ExternalInput")  # bq|bk chunks
    bvbc_d = nc.dram_tensor("bvbc", [P, CPC], F32, kind="ExternalInput")
    wfc_d = nc.dram_tensor("wfc", [16, P, KE * P], BF16, kind="ExternalInput")
    vecs_d = nc.dram_tensor("vecs", [P, 6 * KE], F32, kind="ExternalInput")
    # vecs: [bfc | g1 | be1 | b2 | g2 | be2] each [P, KE]
    w1_d = nc.dram_tensor("w1", [64, P, KE * P], BF16, kind="ExternalInput")
    b1_d = nc.dram_tensor("b1", [P, 64], F32, kind="ExternalInput")
    w2_d = nc.dram_tensor("w2", [4, 16, P, 16 * P], BF16, kind="ExternalInput")
    mask_d = nc.dram_tensor("maskT", [P, 4 * 512], BF16, kind="ExternalInput")
    ones_d = nc.dram_tensor("onesblk", [P, P], BF16, kind="ExternalInput")
    onesf_d = nc.dram_tensor("onesfblk", [P, P], F32R, kind="ExternalInput")

    out_d = nc.dram_tensor("outp", [P, KE, TBLK], F32, kind="ExternalOutput")

    # ---- internal DRAM ----
    qT_d = nc.dram_tensor("qT_i", [CPC, T], BF16, kind="Internal")
    kT_d = nc.dram_tensor("kT_i", [CPC, T], BF16, kind="Internal")
    v_d = nc.dram_tensor("v_i", [T, CPC], F32R, kind="Internal")
    HB = TBLK // 2   # 256-token half-block
    a2a0in_d = nc.dram_tensor("a2a0in_i", [N_CORES, CPC, HB], BF16, kind="Internal")
    a2a0out_d = nc.dram_tensor("a2a0out_i", [N_CORES, CPC, HB], BF16, kind="Internal")
    a2a1in_d = nc.dram_tensor("a2a1in_i", [N_CORES, CPC, HB], BF16, kind="Internal")
    a2a1out_d = nc.dram_tensor("a2a1out_i", [N_CORES, CPC, HB], BF16, kind="Internal")

    with tile.TileContext(nc) as tc:
        with (
            tc.tile_pool(name="const", bufs=1) as cpool,
            tc.tile_pool(name="persist", bufs=1) as ppool,
        ):
            # the q-weight/embedding interleave below goes FIRST on the sync
            # queue so the first matmul can start ASAP; consts follow.
            ones = cpool.tile([P, P], BF16, name="ones")
            onesf = cpool.tile([P, P], F32R, name="onesf")
            mask_t = cpool.tile([P, 4, 512], BF16, name="mask_t")
            nc.scalar.dma_start(mask_t[:], mask_d[:].rearrange("p (f t) -> p f t", f=4))
            bqk_t = cpool.tile([P, 4], F32, name="bqk_t")
            nc.scalar.dma_start(bqk_t[:], bqk_d[:])
            bvbc_t = cpool.tile([P, CPC], F32, name="bvbc_t")
            nc.scalar.dma_start(bvbc_t[:], bvbc_d[:])
            vecs_t = cpool.tile([P, 6, KE], F32, name="vecs_t")
            nc.scalar.dma_start(vecs_t[:], vecs_d[:].rearrange("p (v k) -> p v k", v=6))
            b1_t = cpool.tile([P, 64], F32, name="b1_t")
            nc.scalar.dma_start(b1_t[:], b1_d[:])
            eps_t = cpool.tile([1, 1], F32, name="eps_t")
            nc.vector.memset(eps_t[:], EPS)

            bfc_t = vecs_t[:, 0, :]
            g1_t = vecs_t[:, 1, :]
            be1_t = vecs_t[:, 2, :]
            b2_t = vecs_t[:, 3, :]
            g2_t = vecs_t[:, 4, :]
            be2_t = vecs_t[:, 5, :]

            old_t = ppool.tile([P, KE, TBLK], BF16, name="old_t")   # LN1 output
            wfc_t = ppool.tile([P, 6, KE, P], BF16, name="wfc_lo")
            ctxL0 = ppool.tile([P, KE, HB], BF16, name="ctxL0")
            ctxL1 = ppool.tile([P, KE, HB], BF16, name="ctxL1")

            # ================= Phase Q: q/k/v projections =================
            with tc.tile_pool(name="qw", bufs=1) as qw:
                wq_t = qw.tile([P, KE, CPC], BF16, name="wq_t")
                wk_t = qw.tile([P, KE, CPC], BF16, name="wk_t")
                wv_t = qw.tile([P, KE, CPC], BF16, name="wv_t")

                def qkv_phase(b, qio, qps, tiles, e_pre=None, first=False):
                    if first:
                        e_pre = qio.tile([P, KE, 512], BF16, tag="emb", bufs=2,
                                         name="e_t")
                        wqv = wq_d[:].rearrange("p (k m) -> p k m", k=KE)
                        e0v = embT_d[:, 0:512].rearrange("(k p) t -> p k t", p=P)
                        for kc in range(0, KE, 4):
                            nc.sync.dma_start(wq_t[:, kc:kc + 4], wqv[:, kc:kc + 4])
                            nc.sync.dma_start(e_pre[:, kc:kc + 4], e0v[:, kc:kc + 4])
                        nc.sync.dma_start(
                            wk_t[:], wk_d[:].rearrange("p (k m) -> p k m", k=KE))
                        nc.sync.dma_start(
                            wv_t[:], wv_d[:].rearrange("p (k m) -> p k m", k=KE))
                        nc.sync.dma_start(ones[:], ones_d[:])
                        nc.sync.dma_start(onesf[:], onesf_d[:])
                        # prefetch the fc weights on the scalar engine's DMA
                        # queue so they stream during Q/A instead of stalling F
                        for nb in range(6):
                            nc.scalar.dma_start(
                                wfc_t[:, nb],
                                wfc_d.ap()[nb].rearrange("p (k m) -> p k m", k=KE))

                    for tbl in range(4):
                        tb = 4 * b + tbl
                        if tbl == 0 and e_pre is not None:
                            e_t = e_pre
                        else:
                            e_t = qio.tile([P, KE, 512], BF16, tag="emb", bufs=2,
                                           name="e_t")
                            nc.sync.dma_start(
                                e_t[:],
                                embT_d[:, tb * 512:(tb + 1) * 512]
                                .rearrange("(k p) t -> p k t", p=P),
                            )
                        for wi, (wt, dst) in enumerate(((wq_t, qT_d), (wk_t, kT_d))):
                            for hc in range(2):
                                pqk = qps.tile([P, 512], F32, tag="pqk", bufs=3,
                                               name="pqk")
                                for k in range(KE):
                                    nc.tensor.matmul(
                                        pqk[:], wt[:, k, hc * P:(hc + 1) * P],
                                        e_t[:, k, :],
                                        start=(k == 0), stop=(k == KE - 1),
                                    )
                                st = qio.tile([P, 512], BF16, tag="qkst", bufs=2,
                                              name="st")
                                nc.scalar.activation(
                                    st[:], pqk[:], Identity,
                                    bias=bqk_t[:, 2 * wi + hc:2 * wi + hc + 1])
                                nc.sync.dma_start(
                                    dst.ap()[hc * P:(hc + 1) * P,
                                             tb * 512:(tb + 1) * 512],
                                    st[:])
                        for tt in range(4):
                            pv = qps.tile([P, CPC], F32, tag="pv", bufs=3, name="pv")
                            for k in range(KE):
                                nc.tensor.matmul(
                                    pv[:], e_t[:, k, tt * P:(tt + 1) * P], wv_t[:, k, :],
                                    start=(k == 0), stop=(k == KE - 1),
                                )
                            vst = qio.tile([P, CPC], F32R, tag="vst", bufs=2, name="vst")
                            with nc.allow_low_precision(reason="f32r v store"):
                                nc.vector.tensor_add(vst[:], pv[:], bvbc_t[:])
                            nc.sync.dma_start(
                                v_d.ap()[tb * 512 + tt * P: tb * 512 + (tt + 1) * P, :],
                                vst[:])
                        # emit this block's attention loads NOW: their DRAM
                        # deps cover only the stores emitted so far, so each
                        # slice streams in as soon as this block's stores
                        # land instead of waiting for the whole phase
                        ts = slice(b * S + tbl * 512, b * S + (tbl + 1) * 512)
                        ls = slice(tbl * 512, (tbl + 1) * 512)
                        for hc in range(2):
                            q_t, k_t, v_t = tiles[hc]
                            nc.gpsimd.dma_start(
                                q_t[:, ls], qT_d.ap()[hc * P:(hc + 1) * P, ts])
                            nc.gpsimd.dma_start(
                                k_t[:, ls], kT_d.ap()[hc * P:(hc + 1) * P, ts])
                            nc.gpsimd.dma_start(
                                v_t[:, 4 * tbl:4 * tbl + 4, :],
                                v_d.ap()[ts, hc * P:(hc + 1) * P]
                                .rearrange("(j p) d -> p j d", p=P),
                            )

                # ============ Phase A: causal attention (per batch) ========
                # scoresT/ctxT per head, all transposed; softmax denom via
                # ones-matmul; sc emission pipelined 2 deep; per-q-tile
                # normalization deferred one q-tile so the PE never waits on
                # the DVE chain (except the last tile of the batch, finalized
                # immediately so the all-to-all fires as early as possible).
                def attn_phase(b, tiles, asc, aps):
                    a2ain = a2a0in_d if b == 0 else a2a1in_d
                    pending = [None]

                    def finalize(st):
                        hc, qt, ctx_ps, l_ps = st
                        l_sb = asc.tile([1, 512], F32, tag="lsb", bufs=2,
                                        name="l_sb")
                        nc.vector.tensor_copy(l_sb[:], l_ps[:])
                        r_sb = asc.tile([1, 512], F32, tag="rsb", bufs=2,
                                        name="r_sb")
                        nc.vector.reciprocal_approx_fast(r_sb[:], l_sb[:])
                        r_r = asc.tile([1, 512], BF16, tag="rr", bufs=2, name="r_r")
                        with nc.allow_low_precision(reason="bf16 recip bcast"):
                            nc.vector.tensor_copy(r_r[:], r_sb[:])
                        rbc_ps = aps.tile([P, 512], F32, tag="sc", bufs=2,
                                          name="rbc_ps")
                        nc.tensor.matmul(rbc_ps[:], ones[0:1, :], r_r[:],
                                         start=True, stop=True)
                        ctx_sb = asc.tile([P, 512], F32, tag="ctxsb", bufs=2,
                                          name="ctx_sb")
                        # on the vector queue (not scalar) so the next tile's
                        # Exp is never queued behind this copy
                        nc.vector.tensor_copy(ctx_sb[:], ctx_ps[:])
                        ctx_f = asc.tile([P, 512], BF16, tag="ctxf", bufs=2,
                                         name="ctx_f")
                        with nc.allow_low_precision(reason="bf16 ctx for a2a"):
                            nc.vector.tensor_mul(ctx_f[:], ctx_sb[:], rbc_ps[:])
                        nc.sync.dma_start(
                            a2ain.ap()[2 * qt, hc * P:(hc + 1) * P, :],
                            ctx_f[:, 0:HB])
                        nc.sync.dma_start(
                            a2ain.ap()[2 * qt + 1, hc * P:(hc + 1) * P, :],
                            ctx_f[:, HB:])

                    for hc in range(2):
                        q_t, k_t, v_t = tiles[hc]
                        for qt in range(4):
                            nkb = 4 * qt + 4
                            ctx_ps = aps.tile([P, 512], F32, tag="ctx", bufs=2,
                                              name="ctx_ps")
                            l_full = aps.tile([P, 512], F32, tag="lr", bufs=2,
                                              name="l_full")
                            l_ps = l_full[0:1, :]
                            ex_pairs = [None] * nkb
                            ex_tiles = [None] * nkb
                            sc_cur = [None]

                            def emit_sc(kb, qt=qt, k_t=k_t, q_t=q_t,
                                        ex_tiles=ex_tiles, sc_cur=sc_cur,
                                        ex_pairs=ex_pairs):
                                # kb-blocks are processed in pairs sharing one
                                # 2-bank psum tile and a single wide Exp.
                                # Causal mask is a 0/1 DVE multiply; the
                                # softmax denominator accumulates on the PE.
                                half = kb % 2
                                if half == 0:
                                    sc_cur[0] = aps.tile([P, 2, 512], F32,
                                                         tag="sc", bufs=2,
                                                         name="sc_ps")
                                sc_ps = sc_cur[0]
                                # causally-dead q columns of diagonal blocks
                                # are never written (stale psum is finite; the
                                # 0/1 mask multiply zeroes exp of it)
                                d = max(0, (kb - 4 * qt)) * P
                                nc.tensor.matmul(
                                    sc_ps[:, half, d:],
                                    k_t[:, kb * P:(kb + 1) * P],
                                    q_t[:, qt * 512 + d:(qt + 1) * 512],
                                    start=True, stop=True)
                                if half == 1:
                                    ex = asc.tile([P, 2, 512], F32R, tag="ex",
                                                  bufs=3, name="ex")
                                    ds = [max(0, kb - 1 - 4 * qt) * P,
                                          max(0, kb - 4 * qt) * P]
                                    any_diag = kb >= 4 * qt
                                    # per-half exp always: the first ctx
                                    # matmul only waits on half 0, and diag
                                    # blocks never read unwritten psum
                                    for hh in (0, 1):
                                        nc.scalar.activation(
                                            ex[:, hh, ds[hh]:],
                                            sc_ps[:, hh, ds[hh]:], Exp)
                                    with nc.allow_low_precision(reason="attn mask"):
                                        for hh in (0, 1):
                                            kbb = kb - 1 + hh
                                            dd = ds[hh]
                                            if kbb >= 4 * qt:
                                                # triangular mask on the
                                                # diagonal 128-col block only
                                                nc.vector.tensor_mul(
                                                    ex[:, hh, dd:dd + P],
                                                    ex[:, hh, dd:dd + P],
                                                    mask_t[:, kbb - 4 * qt,
                                                           dd:dd + P])
                                    ex_pairs[kb] = (ex, any_diag, ds)
                                    ex_tiles[kb - 1] = ex[:, 0, :]
                                    ex_tiles[kb] = ex[:, 1, :]

                            for w in range(min(4, nkb)):
                                emit_sc(w)
                            for kb in range(nkb):
                                if kb + 4 < nkb:
                                    emit_sc(kb + 4)
                                d = max(0, (kb - 4 * qt)) * P
                                nc.tensor.matmul(ctx_ps[:, d:], v_t[:, kb, :],
                                                 ex_tiles[kb][:, d:],
                                                 start=(kb == 0),
                                                 stop=(kb == nkb - 1))
                                ex_tiles[kb] = None
                                if kb % 2 == 1:
                                    # softmax denominator rides the PE,
                                    # clipped to the live range per half
                                    ex, any_diag, ds = ex_pairs[kb]
                                    last = kb == nkb - 1
                                    nc.tensor.matmul(
                                        l_ps[:, ds[0]:], onesf[:, 0:1],
                                        ex[:, 0, ds[0]:],
                                        start=(kb == 1), stop=False)
                                    nc.tensor.matmul(
                                        l_ps[:, ds[1]:], onesf[:, 0:1],
                                        ex[:, 1, ds[1]:],
                                        start=False, stop=last)
                                    ex_pairs[kb] = None
                            if pending[0] is not None:
                                finalize(pending[0])
                            pending[0] = (hc, qt, ctx_ps, l_ps)
                    # last tile: finalize immediately so the all-to-all for
                    # this batch fires as early as possible
                    finalize(pending[0])

                with (
                    tc.tile_pool(name="aio", bufs=1) as aio,
                    tc.tile_pool(name="asc", bufs=1) as asc,
                ):
                    def alloc_tiles():
                        out = {}
                        for hc in range(2):
                            out[hc] = (
                                aio.tile([P, S], BF16, tag="q", bufs=2,
                                         name="q_t"),
                                aio.tile([P, S], BF16, tag="k", bufs=2,
                                         name="k_t"),
                                aio.tile([P, 16, P], F32R, tag="v", bufs=2,
                                         name="v_t"),
                            )
                        return out

                    tiles0 = alloc_tiles()
                    with (
                        tc.tile_pool(name="qio0", bufs=2) as qio0,
                        tc.tile_pool(name="qps0", bufs=1, space="PSUM") as qps0,
                    ):
                        qkv_phase(0, qio0, qps0, tiles0, first=True)
                    with tc.tile_pool(name="qio1", bufs=2) as qio1:
                        with tc.tile_pool(name="aps0", bufs=1,
                                          space="PSUM") as aps0:
                            attn_phase(0, tiles0, asc, aps0)
                        # prefetch batch-1's first embedding block on the
                        # gpsimd queue (idle after the attention loads) so
                        # qkv(b1) starts the moment attention(b0) ends
                        e_pre1 = qio1.tile([P, KE, 512], BF16, tag="emb",
                                           bufs=2, name="e_t")
                        nc.gpsimd.dma_start(
                            e_pre1[:],
                            embT_d[:, 4 * 512:5 * 512]
                            .rearrange("(k p) t -> p k t", p=P))
                        # batch-0 ctx fully written -> exchange it while
                        # batch-1 qkv + attention run.
                        nc.gpsimd.collective_compute(
                            "AllToAll", mybir.AluOpType.bypass,
                            replica_groups=[list(range(N_CORES))],
                            ins=[a2a0in_d.ap()], outs=[a2a0out_d.ap()],
                        )
                        tiles1 = alloc_tiles()
                        with tc.tile_pool(name="qps1", bufs=1,
                                          space="PSUM") as qps1:
                            qkv_phase(1, qio1, qps1, tiles1, e_pre=e_pre1)
                    with tc.tile_pool(name="aps1", bufs=1,
                                      space="PSUM") as aps1:
                        attn_phase(1, tiles1, asc, aps1)
                # ctxL0 load sits AFTER batch-1's q/k/v loads on the gpsimd
                # queue so attention never queues behind a collective wait.
                nc.gpsimd.dma_start(
                    ctxL0[:],
                    a2a0out_d.ap().rearrange("r (c p) t -> p (r c) t", p=P))
                nc.gpsimd.collective_compute(
                    "AllToAll", mybir.AluOpType.bypass,
                    replica_groups=[list(range(N_CORES))],
                    ins=[a2a1in_d.ap()], outs=[a2a1out_d.ap()],
                )
                # on scalar (not gpsimd/sync): those queues' close-drains and
                # the fc embres stream must not wait on this collective-gated
                # DMA; scalar is idle until well after the exchange lands
                nc.scalar.dma_start(
                    ctxL1[:],
                    a2a1out_d.ap().rearrange("r (c p) t -> p (r c) t", p=P))

            # ====== Phase F: fc with full Wfc + residual + LN1 stats =======
            # nwp (FFN weight/hidden tiles) opens BEFORE fio so its tiles
            # don't alias x_t: the first w1 loads then stream during fc
            # instead of waiting for LN1 to release x_t's memory.
            with tc.tile_pool(name="nwp", bufs=1) as nwp:
                y_sb = nwp.tile([P, KE, TBLK], F32, name="y_sb")  # FFN accum
                with (
                    tc.tile_pool(name="fio", bufs=1) as fio,
                    tc.tile_pool(name="fps", bufs=1, space="PSUM") as fps,
                ):
                    x_t = fio.tile([P, KE, TBLK], BF16, name="x_t")
                    mu_ps = fps.tile([1, 512], F32, tag="ln1_mu", bufs=1,
                                     name="ln1_mu")
                    sq_ps = fps.tile([1, 512], F32, tag="ln1_sq", bufs=1,
                                     name="ln1_sq")

                    def fc_stats(nb, cs):
                        nc.tensor.matmul(mu_ps[:, cs], ones[:, 0:1],
                                         x_t[:, nb, cs],
                                         start=(nb == 0), stop=(nb == 15))
                        sqk = fio.tile([P, HB], BF16, tag="sqk", bufs=3,
                                       name="sqk")
                        with nc.allow_low_precision(reason="bf16 ln1 squares"):
                            nc.vector.tensor_mul(sqk[:], x_t[:, nb, cs],
                                                 x_t[:, nb, cs])
                        nc.tensor.matmul(sq_ps[:, cs], ones[:, 0:1], sqk[:],
                                         start=(nb == 0), stop=(nb == 15))

                    wfc_hi = {}
                    ln1_pending = [None]
                    for h in range(2):
                        cs = slice(h * HB, (h + 1) * HB)
                        ctxh = ctxL0 if h == 0 else ctxL1
                        for nb in range(16):
                            if (h == 1 and nb == 4
                                    and ln1_pending[0] is not None):
                                # deferred so the PE never waits on the
                                # ln1-h0 DVE chain between the fc halves
                                ln1_pending[0]()
                                ln1_pending[0] = None
                            if nb < 6:
                                wnb = wfc_t[:, nb]
                            elif nb < 10:
                                # cached across both column halves
                                if h == 0:
                                    whi = fio.tile([P, KE, P], BF16,
                                                   tag="wfchi", bufs=4,
                                                   name="wfc_hi")
                                    nc.sync.dma_start(
                                        whi[:],
                                        wfc_d.ap()[nb].rearrange(
                                            "p (k m) -> p k m", k=KE))
                                    wfc_hi[nb] = whi
                                wnb = wfc_hi[nb][:]
                            else:
                                # re-streamed per half (SBUF pressure)
                                whi = fio.tile([P, KE, P], BF16, tag="wfcs",
                                               bufs=2, name="wfc_s")
                                nc.sync.dma_start(
                                    whi[:],
                                    wfc_d.ap()[nb].rearrange(
                                        "p (k m) -> p k m", k=KE))
                                wnb = whi[:]
                            embres_t = fio.tile([P, HB], BF16, tag="embres",
                                                bufs=3, name="embres_t")
                            nc.sync.dma_start(
                                embres_t[:],
                                embres_d[:, nb * TBLK + h * HB:
                                         nb * TBLK + (h + 1) * HB])
                            pfc = fps.tile([P, HB], F32, tag="pfc", bufs=3,
                                           name="pfc")
                            for k in range(KE):
                                nc.tensor.matmul(pfc[:], wnb[:, k, :],
                                                 ctxh[:, k, :],
                                                 start=(k == 0),
                                                 stop=(k == KE - 1))
                            with nc.allow_low_precision(reason="bf16 ln1 input"):
                                nc.vector.scalar_tensor_tensor(
                                    x_t[:, nb, cs], pfc[:], bfc_t[:, nb:nb + 1],
                                    embres_t[:], ADD, ADD)
                            if nb > 0:
                                fc_stats(nb - 1, cs)
                        fc_stats(15, cs)

                        def ln1_fin(cs=cs):
                            _ln_finish(nc, fio, fps, x_t, onesf, g1_t, be1_t,
                                       eps_t, old_t, "ln1", mu_ps=mu_ps,
                                       sq_ps=sq_ps, cs=cs)
                        if h == 0:
                            ln1_pending[0] = ln1_fin
                        else:
                            ln1_fin()

                # ======== Phase N: FFN with LN2 input + stats fused ========
                with tc.tile_pool(name="l2", bufs=1) as l2p:
                    x2_t = l2p.tile([P, KE, TBLK], F32R, name="x2_t")
                    mu2_sb = l2p.tile([1, 512], F32R, name="mu2_sb")
                    m22_sb = l2p.tile([1, 512], F32, name="m22_sb")
                    with tc.tile_pool(name="nps", bufs=1, space="PSUM") as nps:
                        mu2_ps = nps.tile([1, 512], F32, tag="ln2_mu", bufs=1,
                                          name="ln2_mu")
                        sq2_ps = nps.tile([1, 512], F32, tag="ln2_sq", bufs=1,
                                          name="ln2_sq")

                        class _HpsShim:
                            """_ln_finish psum allocator that reuses the idle
                            hps tag so ln2 can run inside the nps scope."""
                            def tile(self, shape, dtype, tag=None, bufs=1,
                                     name=None):
                                return nps.tile(shape, dtype, tag="hps",
                                                bufs=3, name=name)
                        hps_shim = _HpsShim()

                        def y_accum(hbg, nb, c2, h_t):
                            w2_t = nwp.tile([P, 16, P], BF16, tag="w2", bufs=2,
                                            name="w2_t")
                            nc.sync.dma_start(
                                w2_t[:],
                                w2_d.ap()[hbg, nb].rearrange(
                                    "p (l m) -> p l m", l=16))
                            yps = nps.tile([P, 512], F32, tag="yps", bufs=3,
                                           name="yps")[:, 0:c2.stop - c2.start]
                            for hl in range(16):
                                nc.tensor.matmul(yps, w2_t[:, hl, :],
                                                 h_t[:, hl, c2],
                                                 start=(hl == 0),
                                                 stop=(hl == 15))
                            return yps

                        for hbg in range(4):
                            h_t = nwp.tile([P, 16, TBLK], BF16, tag="h",
                                           bufs=1, name="h_t")
                            if hbg == 0:
                                # bridge: ALL h0-column halves of the first 3
                                # blocks run while ln1-h1 is still applying,
                                # then their h1 halves. Per-half gelu.
                                pre_w1 = []
                                pre_hps = []
                                for hl in range(3):
                                    w1_t = nwp.tile([P, KE, P], BF16,
                                                    tag="w1", bufs=3,
                                                    name="w1_t")
                                    nc.sync.dma_start(
                                        w1_t[:],
                                        w1_d.ap()[hl].rearrange(
                                            "p (k m) -> p k m", k=KE))
                                    pre_w1.append(w1_t)
                                    pre_hps.append(nps.tile(
                                        [P, 512], F32, tag="hps", bufs=3,
                                        name="hps"))
                                for h2 in range(2):
                                    c2 = slice(h2 * HB, (h2 + 1) * HB)
                                    for hl in range(3):
                                        for k in range(KE):
                                            nc.tensor.matmul(
                                                pre_hps[hl][:, c2],
                                                pre_w1[hl][:, k, :],
                                                old_t[:, k, c2],
                                                start=(k == 0),
                                                stop=(k == KE - 1))
                                        nc.scalar.activation(
                                            h_t[:, hl, c2],
                                            pre_hps[hl][:, c2], Gelu,
                                            bias=b1_t[:, hl:hl + 1])
                            for hl in range(3 if hbg == 0 else 0, 16):
                                hb = hbg * 16 + hl
                                w1_t = nwp.tile([P, KE, P], BF16, tag="w1",
                                                bufs=3, name="w1_t")
                                nc.sync.dma_start(
                                    w1_t[:],
                                    w1_d.ap()[hb].rearrange("p (k m) -> p k m",
                                                            k=KE))
                                hps = nps.tile([P, 512], F32, tag="hps",
                                               bufs=3, name="hps")
                                for k in range(KE):
                                    nc.tensor.matmul(hps[:], w1_t[:, k, :],
                                                     old_t[:, k, :],
                                                     start=(k == 0),
                                                     stop=(k == KE - 1))
                                nc.scalar.activation(h_t[:, hl, :], hps[:],
                                                     Gelu,
                                                     bias=b1_t[:, hb:hb + 1])
                            if hbg < 3:
                                for nb in range(16):
                                    yps = y_accum(hbg, nb, slice(0, 512), h_t)
                                    if hbg == 0:
                                        nc.vector.tensor_copy(y_sb[:, nb, :],
                                                              yps)
                                    else:
                                        nc.vector.tensor_add(y_sb[:, nb, :],
                                                             y_sb[:, nb, :],
                                                             yps)
                            else:
                                # last group: split by column half so ln2 on
                                # half 0 overlaps the half-1 matmuls; its
                                # emission is deferred a few nb groups so the
                                # PE never waits on the ln2-h0 DVE chain
                                ln_pending = [None]
                                for h2 in range(2):
                                    c2 = slice(h2 * HB, (h2 + 1) * HB)
                                    for nb in range(16):
                                        if (h2 == 1 and nb == 4
                                                and ln_pending[0] is not None):
                                            ln_pending[0]()
                                            ln_pending[0] = None
                                        yps = y_accum(hbg, nb, c2, h_t)
                                        nc.vector.tensor_add(y_sb[:, nb, c2],
                                                             y_sb[:, nb, c2],
                                                             yps)
                                        nc.vector.scalar_tensor_tensor(
                                            x2_t[:, nb, c2], y_sb[:, nb, c2],
                                            b2_t[:, nb:nb + 1],
                                            old_t[:, nb, c2], ADD, ADD)
                                        sq2 = nwp.tile([P, 512], F32R,
                                                       tag="sq2", bufs=1,
                                                       name="sq2")[:, 0:HB]
                                        nc.vector.tensor_mul(
                                            sq2, x2_t[:, nb, c2],
                                            x2_t[:, nb, c2])
                                        nc.tensor.matmul(mu2_ps[:, c2],
                                                         onesf[:, 0:1],
                                                         x2_t[:, nb, c2],
                                                         start=(nb == 0),
                                                         stop=(nb == 15))
                                        nc.tensor.matmul(sq2_ps[:, c2],
                                                         onesf[:, 0:1], sq2,
                                                         start=(nb == 0),
                                                         stop=(nb == 15))
                                    nc.scalar.activation(mu2_sb[:, c2],
                                                         mu2_ps[:, c2], Copy,
                                                         scale=1.0 / E)
                                    nc.scalar.activation(m22_sb[:, c2],
                                                         sq2_ps[:, c2], Copy,
                                                         scale=1.0 / E)

                                    def fin(c2=c2):
                                        _ln_finish(
                                            nc, l2p, hps_shim, x2_t, onesf,
                                            g2_t, be2_t, eps_t,
                                            lambda k: l2p.tile(
                                                [P, 512], F32, tag="osb",
                                                bufs=2, name="out_sb")[:, 0:HB],
                                            "ln2", mu_sb=mu2_sb[:, c2],
                                            m2_sb=m22_sb[:, c2],
                                            chunk_done=lambda k, ap, c2=c2:
                                                nc.sync.dma_start(
                                                    out_d.ap()[:, k, c2], ap),
                                            cs=c2)
                                    if h2 == 0:
                                        ln_pending[0] = fin
                                    else:
                                        fin()

    nc.compile()
    return nc


@functools.lru_cache(maxsize=1)
def _get_program():
    return _build_program()


def _pack_w(w):
    """[E_rows, M] -> [128, (E_rows/128)*M] with [p, k, m] layout."""
    e, m = w.shape
    return np.ascontiguousarray(
        w.reshape(e // P, P, m).transpose(1, 0, 2).reshape(P, -1))


def _pack_vec(v):
    """[n*128] -> [128, n] per-partition chunks."""
    return np.ascontiguousarray(v.reshape(-1, P).T)


def _prepare_in_maps(inputs):
    f32 = np.float32
    emb = np.asarray(inputs["embeddings"], f32).reshape(T, E)
    embT = np.ascontiguousarray(emb.T.astype(BF16NP))
    scale = 1.0 / math.sqrt(HD)

    Wq = np.asarray(inputs["Wq"], f32)
    Wk = np.asarray(inputs["Wk"], f32)
    Wv = np.asarray(inputs["Wv"], f32)
    bq = np.asarray(inputs["bq"], f32)
    bk = np.asarray(inputs["bk"], f32)
    bv = np.asarray(inputs["bv"], f32)
    Wfc = np.asarray(inputs["Wfc"], f32)
    W1 = np.asarray(inputs["W1"], f32)
    W2 = np.asarray(inputs["W2"], f32)

    vecs = np.concatenate([
        _pack_vec(np.asarray(inputs[n], f32))
        for n in ("bfc", "g1", "be1", "b2", "g2", "be2")
    ], axis=1)  # [128, 6*KE]

    wfcp = np.ascontiguousarray(
        Wfc.reshape(KE, P, 16, P).transpose(2, 1, 0, 3).reshape(16, P, KE * P)
        .astype(BF16NP))
    w1p = np.ascontiguousarray(
        W1.reshape(KE, P, 64, P).transpose(2, 1, 0, 3).reshape(64, P, KE * P)
        .astype(BF16NP))
    w2p = np.ascontiguousarray(
        W2.reshape(4, 16, P, 16, P).transpose(0, 3, 2, 1, 4).reshape(4, 16, P, 16 * P)
        .astype(BF16NP))
    b1p = np.ascontiguousarray(np.asarray(inputs["b1"], f32).reshape(64, P).T)

    j = np.arange(P)[:, None, None]
    pp = np.arange(4)[None, :, None]
    cc = np.arange(512)[None, None, :]
    maskT = np.where(P * pp + j <= cc, 1.0, 0.0).astype(BF16NP).reshape(P, 4 * 512)
    onesblk = np.ones((P, P), BF16NP)
    onesfblk = np.ones((P, P), f32)

    in_maps = []
    for c in range(N_CORES):
        sl = slice(CPC * c, CPC * (c + 1))
        bqs = (bq[sl] * scale).reshape(2, P).T
        bks = bk[sl].reshape(2, P).T
        in_maps.append({
            "embT": embT,
            "embres": np.ascontiguousarray(
                np.concatenate(
                    [embT[:, 256 * c:256 * (c + 1)],
                     embT[:, S + 256 * c:S + 256 * (c + 1)]], axis=1)
                .reshape(KE, P, TBLK).transpose(1, 0, 2).reshape(P, KE * TBLK)),
            "wq": _pack_w(Wq[:, sl] * scale).astype(BF16NP),
            "wk": _pack_w(Wk[:, sl]).astype(BF16NP),
            "wv": _pack_w(Wv[:, sl]).astype(BF16NP),
            "bqk": np.ascontiguousarray(np.concatenate([bqs, bks], axis=1)),
            "bvbc": np.ascontiguousarray(np.broadcast_to(bv[sl], (P, CPC))),
            "wfc": wfcp,
            "vecs": vecs,
            "w1": w1p,
            "b1": b1p,
            "w2": w2p,
            "maskT": maskT,
            "onesblk": onesblk,
            "onesfblk": onesfblk,
        })
    return in_maps


def kernel(**inputs) -> np.ndarray:
    nc = _get_program()
    in_maps = _prepare_in_maps(inputs)
    res = None
    last_err = None
    for attempt in range(3):
        try:
            res = run_bass_kernel_spmd(nc, in_maps, core_ids=list(range(N_CORES)))
            break
        except Exception as e:  # transient device/runtime hiccup: retry
            last_err = e
            import time as _time
            _time.sleep(3.0)
    if res is None:
        raise last_err
    out = np.empty((T, E), dtype=np.float32)
    for c in range(N_CORES):
        o = res.results[c]["outp"]          # [128, KE, 512] = [p, k, t]
        sl = o.transpose(1, 0, 2).reshape(E, TBLK)   # [E, 512]
        out[256 * c:256 * (c + 1)] = sl[:, 0:256].T
        out[S + 256 * c:S + 256 * (c + 1)] = sl[:, 256:].T
    return np.ascontiguousarray(out.reshape(B, S, E))
